# revision 2
# baseline (speedup 1.0000x reference)
"""CGCNN forward on 8 Trainium2 NeuronCores — conv layers fully on-device.

Layout: transposed (features on partitions, entities on free), fp16 data.
Edges sorted by dst; core k owns nodes [k*NPAD/8, (k+1)*NPAD/8) and all
edges into them, grouped 128-per-128-node-window (uniform GPW groups per
window for SPMD). h_n replicated; per-layer: AllReduce of BN stats (1KB) +
AllGather of agg slices. Node embedding and graph pooling/head on host.
"""
import sys
sys.path.insert(0, "/opt/trn_rl_repo")
import numpy as np

EPS = 1e-5
NODE_F, EDGE_F, FEAT, NCONV = 92, 41, 64, 3

# problem sizes (overridable for mini tests)
N, E, G = 25000, 400000, 128
NCORES = 8
NPAD = 25600

_cache = {}


def _derived():
    NW = NPAD // 128 // NCORES
    NSLICE = NPAD // NCORES
    ZROW = NPAD
    TROWS = NPAD + 16
    return NW, NSLICE, ZROW, TROWS


# ----------------------------------------------------------------- host prep
def _host_prep(src, dst):
    NW, NSLICE, ZROW, TROWS = _derived()
    order = np.argsort(dst, kind="stable")
    dsts = dst[order]
    srcs = src[order]
    nwin = NPAD // 128
    win = dsts // 128
    wcnt = np.bincount(win, minlength=nwin)
    GPW = int(np.max((wcnt + 127) // 128))
    NGRP = NW * GPW
    EPAD = ((NGRP * 128 + 2047) // 2048) * 2048
    wstart = np.concatenate([[0], np.cumsum(wcnt)])
    src_idx = np.full((NCORES, EPAD), ZROW, np.int16)
    dst_idx = np.full((NCORES, EPAD), ZROW, np.int16)
    dloc = np.full((NCORES, NGRP * 128), -1.0, np.float32)
    eperm = np.full((NCORES, EPAD), -1, np.int64)
    for k in range(NCORES):
        for w in range(NW):
            gw = k * NW + w
            a, b = wstart[gw], wstart[gw + 1]
            ne = b - a
            base = w * GPW * 128
            src_idx[k, base:base + ne] = srcs[a:b].astype(np.int16)
            dst_idx[k, base:base + ne] = dsts[a:b].astype(np.int16)
            dloc[k, base:base + ne] = (dsts[a:b] - gw * 128).astype(np.float32)
            eperm[k, base:base + ne] = order[a:b]

    def wrap16(idx2d):
        out = np.zeros((NCORES, 128, EPAD // 16), np.int16)
        for k in range(NCORES):
            blk = idx2d[k].reshape(EPAD // 16, 16).T
            for c in range(8):
                out[k, c * 16:(c + 1) * 16, :] = blk
        return out

    # dloc per-partition layout: [128 edge-in-group, NGRP]
    dloc_pp = dloc.reshape(NCORES, NGRP, 128).transpose(0, 2, 1).copy()
    return dict(GPW=GPW, EPAD=EPAD, NGRP=NGRP, src_w=wrap16(src_idx),
                dst_w=wrap16(dst_idx), dloc=dloc_pp, eperm=eperm)


# ------------------------------------------------------------ module builder
def _build(EPAD, GPW):
    import concourse.bacc as bacc
    import concourse.mybir as mybir
    import concourse.tile as tile
    from concourse.masks import make_identity

    NW, NSLICE, ZROW, TROWS = _derived()
    f16, f32 = mybir.dt.float16, mybir.dt.float32
    AF = mybir.ActivationFunctionType
    OP = mybir.AluOpType
    X = mybir.AxisListType.X
    NGRP = NW * GPW
    CH = 512
    NCH = EPAD // CH
    GCH = 2048
    NGC = EPAD // GCH
    assert EPAD % GCH == 0 and EPAD % CH == 0 and EPAD >= NGRP * 128
    RG = [[i for i in range(NCORES)]]

    nc = bacc.Bacc("TRN2", target_bir_lowering=False, debug=False,
                   num_devices=NCORES)
    dt_ = nc.dram_tensor
    efT = dt_("efT", [EDGE_F, EPAD], f16, kind="ExternalInput")
    hnT0 = dt_("hnT0", [FEAT, NPAD], f16, kind="ExternalInput")
    srcw = dt_("srcw", [128, EPAD // 16], mybir.dt.int16, kind="ExternalInput")
    dstw = dt_("dstw", [128, EPAD // 16], mybir.dt.int16, kind="ExternalInput")
    dlocd = dt_("dlocd", [128, NGRP], f32, kind="ExternalInput")
    w_ee = dt_("w_ee", [EDGE_F, FEAT], f16, kind="ExternalInput")
    w_emg = dt_("w_emg", [FEAT, NCONV * 128], f16, kind="ExternalInput")
    w_cat = dt_("w_cat", [FEAT, NCONV * 256], f16, kind="ExternalInput")
    gb_e = dt_("gb_e", [FEAT, 2], f32, kind="ExternalInput")
    gb_mg = dt_("gb_mg", [128, NCONV * 2], f32, kind="ExternalInput")
    gb_n = dt_("gb_n", [FEAT, NCONV * 2], f32, kind="ExternalInput")
    npadv = dt_("npadv", [128, 1], f32, kind="ExternalInput")
    hnT_out = dt_("hnT_out", [FEAT, NPAD], f16, kind="ExternalOutput")
    tbl = dt_("tbl", [TROWS, 256], f16)
    zeD = dt_("zeD", [FEAT, EPAD], f16)
    heD = dt_("heD", [FEAT, EPAD], f16)
    cc_in = dt_("cc_in", [128, 2], f32)
    cc_out = dt_("cc_out", [128, 2], f32)
    ag_in = dt_("ag_in", [FEAT * NSLICE], f16)
    ag_out = dt_("ag_out", [NCORES * FEAT * NSLICE], f16)

    def allreduce():
        if NCORES == 1:
            nc.sync.dma_start(cc_out[:], cc_in[:])
        else:
            nc.gpsimd.collective_compute(
                "AllReduce", OP.add, replica_groups=RG,
                ins=[cc_in[:].opt()], outs=[cc_out[:].opt()])

    def allgather():
        if NCORES == 1:
            nc.sync.dma_start(ag_out[:], ag_in[:])
        else:
            nc.gpsimd.collective_compute(
                "AllGather", OP.bypass, replica_groups=RG,
                ins=[ag_in[:].opt()], outs=[ag_out[:].opt()])

    with tile.TileContext(nc) as tc:
        with tc.tile_pool(name="persist", bufs=1) as pp:
            hnT = pp.tile([FEAT, NPAD], f16)
            zmg = pp.tile([128, EPAD], f16)
            srcw_s = pp.tile([128, EPAD // 16], mybir.dt.int16)
            dstw_s = pp.tile([128, EPAD // 16], mybir.dt.int16)
            dloc_s = pp.tile([128, NGRP], f32)
            w_ee_s = pp.tile([EDGE_F, FEAT], f16)
            w_emg_s = pp.tile([FEAT, NCONV * 128], f16)
            w_cat_s = pp.tile([FEAT, NCONV * 256], f16)
            gb_e_s = pp.tile([FEAT, 2], f32)
            gb_mg_s = pp.tile([128, NCONV * 2], f32)
            gb_n_s = pp.tile([FEAT, NCONV * 2], f32)
            npad_s = pp.tile([128, 1], f32)
            ident = pp.tile([128, 128], f16)
            iota_row = pp.tile([128, 128], f32)
            sring = pp.tile([128, 2 * NCH + 2], f32)
            st = pp.tile([128, 8], f32)
            sc_m = pp.tile([128, 1], f32)
            sc_t = pp.tile([128, 1], f32)
            eps_t = pp.tile([128, 1], f32)
            nc.vector.memset(eps_t[:], EPS)
            hpad16 = pp.tile([FEAT, 1], f16)
            cpv = pp.tile([128, 1], f32)
            cp2 = pp.tile([128, 1], f32)

            nc.sync.dma_start(hnT[:], hnT0[:])
            nc.sync.dma_start(srcw_s[:], srcw[:])
            nc.sync.dma_start(dstw_s[:], dstw[:])
            nc.sync.dma_start(dloc_s[:], dlocd[:])
            nc.sync.dma_start(w_ee_s[:], w_ee[:])
            nc.sync.dma_start(w_emg_s[:], w_emg[:])
            nc.sync.dma_start(w_cat_s[:], w_cat[:])
            nc.sync.dma_start(gb_e_s[:], gb_e[:])
            nc.sync.dma_start(gb_mg_s[:], gb_mg[:])
            nc.sync.dma_start(gb_n_s[:], gb_n[:])
            nc.sync.dma_start(npad_s[:], npadv[:])
            make_identity(nc, ident[:])
            ii = pp.tile([128, 128], mybir.dt.int32)
            nc.gpsimd.iota(ii[:], pattern=[[1, 128]], base=0,
                           channel_multiplier=0)
            nc.vector.tensor_copy(iota_row[:], ii[:])

            def bn_affine(p, g_ap, b_ap, inv_n):
                """st[:p,0:2] holds (sum, sumsq); writes sc_m/sc_t[:p]."""
                nc.vector.tensor_scalar(out=st[:p, 2:3], in0=st[:p, 0:1],
                                        scalar1=inv_n, scalar2=None,
                                        op0=OP.mult)
                nc.vector.tensor_scalar(out=st[:p, 3:4], in0=st[:p, 1:2],
                                        scalar1=inv_n, scalar2=None,
                                        op0=OP.mult)
                nc.vector.tensor_tensor(out=st[:p, 4:5], in0=st[:p, 2:3],
                                        in1=st[:p, 2:3], op=OP.mult)
                nc.vector.tensor_tensor(out=st[:p, 3:4], in0=st[:p, 3:4],
                                        in1=st[:p, 4:5], op=OP.subtract)
                nc.scalar.activation(st[:p, 3:4], st[:p, 3:4], AF.Sqrt,
                                     bias=eps_t[:p, :])
                nc.vector.reciprocal(st[:p, 3:4], st[:p, 3:4])
                nc.vector.tensor_tensor(out=sc_m[:p, :], in0=st[:p, 3:4],
                                        in1=g_ap, op=OP.mult)
                nc.vector.tensor_tensor(out=st[:p, 5:6], in0=sc_m[:p, :],
                                        in1=st[:p, 2:3], op=OP.mult)
                nc.vector.tensor_tensor(out=sc_t[:p, :], in0=b_ap,
                                        in1=st[:p, 5:6], op=OP.subtract)

            # ---------------- phase E: z = W_ee.T @ efT, stats, silu
            with tc.tile_pool(name="peb", bufs=3) as sb, \
                 tc.tile_pool(name="pep", bufs=2, space="PSUM") as ps:
                for c in range(NCH):
                    x = sb.tile([EDGE_F, CH], f16, tag="x")
                    nc.sync.dma_start(x[:], efT[:, c * CH:(c + 1) * CH])
                    z = ps.tile([FEAT, CH], f32, tag="z")
                    nc.tensor.matmul(z[:], lhsT=w_ee_s[:], rhs=x[:],
                                     start=True, stop=True)
                    zs = sb.tile([FEAT, CH], f16, tag="zs")
                    nc.scalar.activation(zs[:], z[:], AF.Identity,
                                         accum_out=sring[:FEAT, c:c + 1])
                    sq = sb.tile([FEAT, CH], f16, tag="sq")
                    nc.scalar.activation(
                        sq[:], zs[:], AF.Square,
                        accum_out=sring[:FEAT, NCH + c:NCH + c + 1])
                    nc.sync.dma_start(zeD[:, c * CH:(c + 1) * CH], zs[:])
            nc.vector.tensor_reduce(out=st[:FEAT, 0:1],
                                    in_=sring[:FEAT, 0:NCH], op=OP.add,
                                    axis=X)
            nc.vector.tensor_reduce(out=st[:FEAT, 1:2],
                                    in_=sring[:FEAT, NCH:2 * NCH], op=OP.add,
                                    axis=X)
            cci = pp.tile([128, 2], f32)
            nc.vector.memset(cci[:], 0.0)
            nc.vector.tensor_copy(cci[:FEAT, :], st[:FEAT, 0:2])
            nc.sync.dma_start(cc_in[:], cci[:])
            allreduce()
            cco = pp.tile([128, 2], f32)
            nc.sync.dma_start(cco[:], cc_out[:])
            nc.vector.tensor_copy(st[:FEAT, 0:2], cco[:FEAT, :])
            bn_affine(FEAT, gb_e_s[:, 0:1], gb_e_s[:, 1:2], 1.0 / E)
            with tc.tile_pool(name="pe2", bufs=3) as sb:
                for c in range(NCH):
                    zl = sb.tile([FEAT, CH], f16, tag="zl")
                    nc.sync.dma_start(zl[:], zeD[:, c * CH:(c + 1) * CH])
                    a = sb.tile([FEAT, CH], f16, tag="a")
                    nc.vector.tensor_scalar(
                        out=a[:], in0=zl[:], scalar1=sc_m[:FEAT, :],
                        scalar2=sc_t[:FEAT, :], op0=OP.mult, op1=OP.add)
                    sg = sb.tile([FEAT, CH], f16, tag="sg")
                    nc.scalar.activation(sg[:], a[:], AF.Sigmoid)
                    h = sb.tile([FEAT, CH], f16, tag="h")
                    nc.vector.tensor_tensor(out=h[:], in0=a[:], in1=sg[:],
                                            op=OP.mult)
                    nc.sync.dma_start(heD[:, c * CH:(c + 1) * CH], h[:])
            # hpad = silu(t) (z=0 for pad cols)
            hpadf = pp.tile([FEAT, 1], f32)
            nc.scalar.activation(hpadf[:], sc_t[:FEAT, :], AF.Sigmoid)
            nc.vector.tensor_tensor(out=hpadf[:], in0=hpadf[:],
                                    in1=sc_t[:FEAT, :], op=OP.mult)
            nc.vector.tensor_copy(hpad16[:], hpadf[:])

            # ---------------- conv layers
            for l in range(NCONV):
                lsl = slice(l * 128, (l + 1) * 128)
                # tables
                with tc.tile_pool(name=f"tb{l}", bufs=3) as sb, \
                     tc.tile_pool(name=f"tp{l}", bufs=2, space="PSUM") as ps:
                    for c in range(NPAD // 128):
                        t0 = ps.tile([128, 256], f32, tag="t0")
                        nc.tensor.matmul(
                            t0[:], lhsT=hnT[:, c * 128:(c + 1) * 128],
                            rhs=w_cat_s[:, l * 256:(l + 1) * 256],
                            start=True, stop=True)
                        stg = sb.tile([128, 256], f16, tag="stg")
                        if c % 2 == 0:
                            nc.vector.tensor_copy(stg[:], t0[:])
                        else:
                            nc.scalar.activation(stg[:], t0[:], AF.Identity)
                        nc.sync.dma_start(tbl[c * 128:(c + 1) * 128, :],
                                          stg[:])
                    if l == 0:
                        zt = sb.tile([16, 256], f16, tag="zt")
                        nc.vector.memset(zt[:], 0.0)
                        nc.sync.dma_start(tbl[NPAD:TROWS, :], zt[:])
                # pad-edge constant
                with tc.tile_pool(name=f"pc{l}", bufs=1, space="PSUM") as ps:
                    cp = ps.tile([128, 1], f32)
                    nc.tensor.matmul(cp[:], lhsT=w_emg_s[:, lsl],
                                     rhs=hpad16[:], start=True, stop=True)
                    nc.vector.tensor_copy(cpv[:], cp[:])
                    nc.vector.tensor_tensor(out=cp2[:], in0=cpv[:],
                                            in1=cpv[:], op=OP.mult)

                # pass1
                with tc.tile_pool(name=f"p1_{l}", bufs=3) as sb, \
                     tc.tile_pool(name=f"g{l}", bufs=2) as gb, \
                     tc.tile_pool(name=f"q{l}", bufs=2, space="PSUM") as ps:
                    for gc in range(NGC):
                        gs = gb.tile([128, 1, GCH], f16, tag="gs")
                        nc.gpsimd.dma_gather(
                            out_ap=gs[:], in_ap=tbl[:, 0:128],
                            idxs_ap=srcw_s[:, gc * (GCH // 16):
                                           (gc + 1) * (GCH // 16)],
                            num_idxs=GCH, num_idxs_reg=GCH, elem_size=128,
                            elem_step=256, transpose=True,
                            single_packet=False)
                        gd = gb.tile([128, 1, GCH], f16, tag="gd")
                        nc.gpsimd.dma_gather(
                            out_ap=gd[:], in_ap=tbl[:, 128:256],
                            idxs_ap=dstw_s[:, gc * (GCH // 16):
                                           (gc + 1) * (GCH // 16)],
                            num_idxs=GCH, num_idxs_reg=GCH, elem_size=128,
                            elem_step=256, transpose=True,
                            single_packet=False)
                        for s in range(GCH // CH):
                            c = gc * (GCH // CH) + s
                            he = sb.tile([FEAT, CH], f16, tag="he")
                            nc.sync.dma_start(he[:],
                                              heD[:, c * CH:(c + 1) * CH])
                            z = ps.tile([128, CH], f32, tag="z")
                            nc.tensor.matmul(z[:], lhsT=w_emg_s[:, lsl],
                                             rhs=he[:], start=True,
                                             stop=False)
                            nc.tensor.matmul(
                                z[:], lhsT=ident[:],
                                rhs=gs[:, 0, s * CH:(s + 1) * CH],
                                start=False, stop=False)
                            nc.tensor.matmul(
                                z[:], lhsT=ident[:],
                                rhs=gd[:, 0, s * CH:(s + 1) * CH],
                                start=False, stop=True)
                            zd = zmg[:, c * CH:(c + 1) * CH]
                            nc.scalar.activation(
                                zd, z[:], AF.Identity,
                                accum_out=sring[:, c:c + 1])
                            sq = sb.tile([128, CH], f16, tag="sq")
                            nc.scalar.activation(
                                sq[:], zd, AF.Square,
                                accum_out=sring[:, NCH + c:NCH + c + 1])
                nc.vector.tensor_reduce(out=st[:, 0:1], in_=sring[:, 0:NCH],
                                        op=OP.add, axis=X)
                nc.vector.tensor_reduce(out=st[:, 1:2],
                                        in_=sring[:, NCH:2 * NCH],
                                        op=OP.add, axis=X)
                nc.vector.tensor_tensor(out=st[:, 2:3], in0=cpv[:],
                                        in1=npad_s[:], op=OP.mult)
                nc.vector.tensor_tensor(out=st[:, 0:1], in0=st[:, 0:1],
                                        in1=st[:, 2:3], op=OP.subtract)
                nc.vector.tensor_tensor(out=st[:, 2:3], in0=cp2[:],
                                        in1=npad_s[:], op=OP.mult)
                nc.vector.tensor_tensor(out=st[:, 1:2], in0=st[:, 1:2],
                                        in1=st[:, 2:3], op=OP.subtract)
                cci2 = pp.tile([128, 2], f32, tag="cci2")
                nc.vector.tensor_copy(cci2[:], st[:, 0:2])
                nc.sync.dma_start(cc_in[:], cci2[:])
                allreduce()
                cco2 = pp.tile([128, 2], f32, tag="cco2")
                nc.sync.dma_start(cco2[:], cc_out[:])
                nc.vector.tensor_copy(st[:, 0:2], cco2[:])
                bn_affine(128, gb_mg_s[:, 2 * l:2 * l + 1],
                          gb_mg_s[:, 2 * l + 1:2 * l + 2], 1.0 / E)

                # pass2
                with tc.tile_pool(name=f"p2_{l}", bufs=2) as sb, \
                     tc.tile_pool(name=f"r2{l}", bufs=2, space="PSUM") as ps, \
                     tc.tile_pool(name=f"a2{l}", bufs=2, space="PSUM") as pa:
                    for c in range(NCH):
                        zc = zmg[:, c * CH:(c + 1) * CH]
                        nc.scalar.activation(zc, zc, AF.Sigmoid,
                                             bias=sc_t[:], scale=sc_m[:])
                    for c in range(NCH):
                        zg = zmg[FEAT:128, c * CH:(c + 1) * CH]
                        nc.scalar.activation(zg, zg, AF.Ln)
                    aggT = pp.tile([FEAT, NSLICE], f16, tag="aggT")
                    QE = 2048
                    ag = None
                    for grp in range(NGRP):
                        e0 = grp * 128
                        if e0 % QE == 0:
                            # move Ln(g) half down to partitions 0-63
                            lb = sb.tile([FEAT, QE], f16, tag="lb")
                            nc.sync.dma_start(
                                lb[:], zmg[FEAT:128, e0:e0 + QE])
                        if e0 % CH == 0:
                            qo = e0 % QE
                            mt = sb.tile([FEAT, CH], f16, tag="mt")
                            nc.vector.tensor_tensor(
                                out=mt[:], in0=zmg[0:FEAT, e0:e0 + CH],
                                in1=lb[:, qo:qo + CH], op=OP.mult)
                        w, g = grp // GPW, grp % GPW
                        if g == 0:
                            ag = pa.tile([FEAT, 128], f32, tag="ag")
                        off = e0 % CH
                        mn = ps.tile([128, FEAT], f16, tag="mn")
                        nc.tensor.transpose(mn[:], mt[:, off:off + 128],
                                            ident[:FEAT, :FEAT])
                        mns = sb.tile([128, FEAT], f16, tag="mns")
                        nc.scalar.activation(mns[:], mn[:], AF.Identity)
                        oh = sb.tile([128, 128], f16, tag="oh")
                        nc.vector.tensor_scalar(
                            out=oh[:], in0=iota_row[:],
                            scalar1=dloc_s[:, grp:grp + 1],
                            scalar2=-1.0, op0=OP.is_equal, op1=OP.mult)
                        nc.tensor.matmul(ag[:], lhsT=mns[:], rhs=oh[:],
                                         start=(g == 0),
                                         stop=(g == GPW - 1))
                        if g == GPW - 1:
                            nc.scalar.activation(
                                aggT[:, w * 128:(w + 1) * 128], ag[:],
                                AF.Identity)
                    nc.sync.dma_start(
                        ag_in[:].rearrange("(a b) -> a b", a=FEAT), aggT[:])
                allgather()
                # agg stats + h_n update
                HS = NSLICE // 2
                NPC = 2 * NCORES  # pieces of [FEAT, HS]
                agr = ag_out[:].rearrange("(c f hh h) -> c f hh h",
                                          c=NCORES, f=FEAT, hh=2)
                with tc.tile_pool(name=f"u{l}", bufs=2) as sb:
                    for k in range(NPC):
                        t = sb.tile([FEAT, HS], f16, tag="agld")
                        nc.sync.dma_start(t[:], agr[k // 2][:, k % 2, :])
                        d1 = sb.tile([FEAT, HS], f16, tag="dsink")
                        nc.scalar.activation(
                            d1[:], t[:], AF.Identity,
                            accum_out=sring[:FEAT, k:k + 1])
                        d2 = sb.tile([FEAT, HS], f16, tag="dsink")
                        nc.scalar.activation(
                            d2[:], t[:], AF.Square,
                            accum_out=sring[:FEAT, NPC + k:NPC + k + 1])
                    nc.vector.tensor_reduce(
                        out=st[:FEAT, 0:1], in_=sring[:FEAT, 0:NPC],
                        op=OP.add, axis=X)
                    nc.vector.tensor_reduce(
                        out=st[:FEAT, 1:2], in_=sring[:FEAT, NPC:2 * NPC],
                        op=OP.add, axis=X)
                    bn_affine(FEAT, gb_n_s[:, 2 * l:2 * l + 1],
                              gb_n_s[:, 2 * l + 1:2 * l + 2], 1.0 / N)
                    for k in range(NPC):
                        t = sb.tile([FEAT, HS], f16, tag="agld")
                        nc.sync.dma_start(t[:], agr[k // 2][:, k % 2, :])
                        tmp = sb.tile([FEAT, HS], f16, tag="tmp")
                        nc.vector.tensor_scalar(
                            out=tmp[:], in0=t[:], scalar1=sc_m[:FEAT, :],
                            scalar2=sc_t[:FEAT, :], op0=OP.mult, op1=OP.add)
                        hsl = hnT[:, k * HS:(k + 1) * HS]
                        nc.vector.tensor_tensor(out=tmp[:], in0=tmp[:],
                                                in1=hsl, op=OP.add)
                        nc.scalar.activation(hsl, tmp[:], AF.Sigmoid)
            nc.sync.dma_start(hnT_out[:], hnT[:])
    nc.compile()
    return nc


# ------------------------------------------------------------------- kernel
def _silu(x):
    return x / (1.0 + np.exp(-x))


def _bn(x, g, b):
    return g * (x - x.mean(0)) / np.sqrt(x.var(0) + EPS) + b


def make_in_maps(inputs, prep):
    """Host-side marshaling: returns (in_maps, host_ctx)."""
    f32 = lambda k: np.asarray(inputs[k], np.float32)
    node_feats = f32("node_feats")
    edge_feats = f32("edge_feats")
    EPAD = prep["EPAD"]

    h_n0 = _silu(_bn(node_feats @ f32("W_ne"), f32("g_ne"), f32("be_ne")))
    hnT0 = np.zeros((FEAT, NPAD), np.float16)
    hnT0[:, :N] = h_n0.T.astype(np.float16)

    Wm, Wg = f32("Wm"), f32("Wg")
    w_ee = f32("W_ee").astype(np.float16)
    w_emg = np.concatenate(
        [np.concatenate([Wm[l][2 * FEAT:], Wg[l][2 * FEAT:]], 1)
         for l in range(NCONV)], 1).astype(np.float16)
    w_cat = np.concatenate(
        [np.concatenate([Wm[l][:FEAT], Wg[l][:FEAT],
                         Wm[l][FEAT:2 * FEAT], Wg[l][FEAT:2 * FEAT]], 1)
         for l in range(NCONV)], 1).astype(np.float16)
    gb_e = np.ascontiguousarray(
        np.stack([f32("g_ee"), f32("be_ee")], 1).astype(np.float32))
    gb_mg = np.zeros((128, NCONV * 2), np.float32)
    gb_n = np.zeros((FEAT, NCONV * 2), np.float32)
    for l in range(NCONV):
        gb_mg[:FEAT, 2 * l] = f32("gm")[l]
        gb_mg[FEAT:, 2 * l] = -f32("gg")[l]
        gb_mg[:FEAT, 2 * l + 1] = f32("bem")[l]
        gb_mg[FEAT:, 2 * l + 1] = -f32("beg")[l]
        gb_n[:, 2 * l] = f32("gn")[l]
        gb_n[:, 2 * l + 1] = f32("ben")[l]

    in_maps = []
    for k in range(NCORES):
        efT = np.zeros((EDGE_F, EPAD), np.float16)
        valid = prep["eperm"][k] >= 0
        efT[:, valid] = edge_feats[prep["eperm"][k][valid]].T.astype(
            np.float16)
        npadv = np.full((128, 1), float(EPAD - valid.sum()), np.float32)
        in_maps.append(dict(
            efT=efT, hnT0=hnT0, srcw=prep["src_w"][k], dstw=prep["dst_w"][k],
            dlocd=np.ascontiguousarray(prep["dloc"][k]), w_ee=w_ee,
            w_emg=w_emg, w_cat=w_cat, gb_e=gb_e, gb_mg=gb_mg, gb_n=gb_n,
            npadv=npadv))
    return in_maps


def head(inputs, hnT):
    f32 = lambda k: np.asarray(inputs[k], np.float32)
    n2g = np.asarray(inputs["node2graph"], np.int64)
    h_n = hnT[:, :N].T.astype(np.float32)
    sums = np.zeros((G, FEAT), np.float32)
    np.add.at(sums, n2g, h_n)
    cnt = np.bincount(n2g, minlength=G).astype(np.float32)[:, None]
    pooled = sums / np.maximum(cnt, 1.0)
    h = _silu(_bn(pooled @ f32("W_fc") + f32("b_fc"), f32("g_fc"),
                  f32("be_fc")))
    return (h @ f32("W_out") + f32("b_out")).astype(np.float32)


def kernel(**inputs):
    import time as _time
    from concourse.bass_utils import run_bass_kernel_spmd

    src = np.asarray(inputs["src"], np.int64)
    dst = np.asarray(inputs["dst"], np.int64)
    prep = _host_prep(src, dst)
    key = ("nc", prep["EPAD"], prep["GPW"])
    if key not in _cache:
        _cache[key] = _build(prep["EPAD"], prep["GPW"])
        try:
            from concourse.timeline_sim import TimelineSim
            globals()["LAST_EXEC_NS"] = int(
                TimelineSim(_cache[key], no_exec=True).simulate())
        except Exception:
            pass
    nc = _cache[key]
    in_maps = make_in_maps(inputs, prep)
    t0 = _time.time()
    res = run_bass_kernel_spmd(nc, in_maps, core_ids=list(range(NCORES)))
    globals()["LAST_WALL_S"] = _time.time() - t0
    hnT = res.results[0]["hnT_out"].astype(np.float32)
    return head(inputs, hnT)


# revision 3
# speedup vs baseline: 1.2884x; 1.2884x over previous
"""CGCNN forward on 8 Trainium2 NeuronCores — conv layers fully on-device.

Layout: transposed (features on partitions, entities on free), fp16 data.
Edges sorted by dst; core k owns nodes [k*NPAD/8, (k+1)*NPAD/8) and all
edges into them, grouped 128-per-128-node-window (uniform GPW groups per
window for SPMD). h_n replicated; per-layer: AllReduce of BN stats (1KB) +
AllGather of agg slices. Node embedding and graph pooling/head on host.
"""
import sys
sys.path.insert(0, "/opt/trn_rl_repo")
import numpy as np

EPS = 1e-5
NODE_F, EDGE_F, FEAT, NCONV = 92, 41, 64, 3

# problem sizes (overridable for mini tests)
N, E, G = 25000, 400000, 128
NCORES = 8
NPAD = 25600

_cache = {}


def _derived():
    NW = NPAD // 128 // NCORES
    NSLICE = NPAD // NCORES
    ZROW = NPAD
    TROWS = NPAD + 16
    return NW, NSLICE, ZROW, TROWS


# ----------------------------------------------------------------- host prep
def _host_prep(src, dst):
    NW, NSLICE, ZROW, TROWS = _derived()
    order = np.argsort(dst, kind="stable")
    dsts = dst[order]
    srcs = src[order]
    nwin = NPAD // 128
    win = dsts // 128
    wcnt = np.bincount(win, minlength=nwin)
    GPW = int(np.max((wcnt + 127) // 128))
    NGRP = NW * GPW
    EPAD = ((NGRP * 128 + 2047) // 2048) * 2048
    wstart = np.concatenate([[0], np.cumsum(wcnt)])
    src_idx = np.full((NCORES, EPAD), ZROW, np.int16)
    dst_idx = np.full((NCORES, EPAD), ZROW, np.int16)
    dloc = np.full((NCORES, NGRP * 128), -1.0, np.float32)
    eperm = np.full((NCORES, EPAD), -1, np.int64)
    for k in range(NCORES):
        for w in range(NW):
            gw = k * NW + w
            a, b = wstart[gw], wstart[gw + 1]
            ne = b - a
            base = w * GPW * 128
            src_idx[k, base:base + ne] = srcs[a:b].astype(np.int16)
            dst_idx[k, base:base + ne] = dsts[a:b].astype(np.int16)
            dloc[k, base:base + ne] = (dsts[a:b] - gw * 128).astype(np.float32)
            eperm[k, base:base + ne] = order[a:b]

    def wrap16(idx2d):
        out = np.zeros((NCORES, 128, EPAD // 16), np.int16)
        for k in range(NCORES):
            blk = idx2d[k].reshape(EPAD // 16, 16).T
            for c in range(8):
                out[k, c * 16:(c + 1) * 16, :] = blk
        return out

    # dloc per-partition layout: [128 edge-in-group, NGRP]
    dloc_pp = dloc.reshape(NCORES, NGRP, 128).transpose(0, 2, 1).copy()
    return dict(GPW=GPW, EPAD=EPAD, NGRP=NGRP, src_w=wrap16(src_idx),
                dst_w=wrap16(dst_idx), dloc=dloc_pp, eperm=eperm)


# ------------------------------------------------------------ module builder
def _build(EPAD, GPW):
    import concourse.bacc as bacc
    import concourse.mybir as mybir
    import concourse.tile as tile
    from concourse.masks import make_identity

    NW, NSLICE, ZROW, TROWS = _derived()
    f16, f32 = mybir.dt.float16, mybir.dt.float32
    AF = mybir.ActivationFunctionType
    OP = mybir.AluOpType
    X = mybir.AxisListType.X
    NGRP = NW * GPW
    CH = 512
    NCH = EPAD // CH
    GCH = 2048
    NGC = EPAD // GCH
    assert EPAD % GCH == 0 and EPAD % CH == 0 and EPAD >= NGRP * 128
    RG = [[i for i in range(NCORES)]]

    nc = bacc.Bacc("TRN2", target_bir_lowering=False, debug=False,
                   num_devices=NCORES)
    dt_ = nc.dram_tensor
    efT = dt_("efT", [EDGE_F, EPAD], f16, kind="ExternalInput")
    hnT0 = dt_("hnT0", [FEAT, NPAD], f16, kind="ExternalInput")
    srcw = dt_("srcw", [128, EPAD // 16], mybir.dt.int16, kind="ExternalInput")
    dstw = dt_("dstw", [128, EPAD // 16], mybir.dt.int16, kind="ExternalInput")
    dlocd = dt_("dlocd", [128, NGRP], f32, kind="ExternalInput")
    w_ee = dt_("w_ee", [EDGE_F, FEAT], f16, kind="ExternalInput")
    w_emg = dt_("w_emg", [FEAT, NCONV * 128], f16, kind="ExternalInput")
    w_cat = dt_("w_cat", [FEAT, NCONV * 256], f16, kind="ExternalInput")
    gb_e = dt_("gb_e", [FEAT, 2], f32, kind="ExternalInput")
    gb_mg = dt_("gb_mg", [128, NCONV * 2], f32, kind="ExternalInput")
    gb_n = dt_("gb_n", [FEAT, NCONV * 2], f32, kind="ExternalInput")
    npadv = dt_("npadv", [128, 1], f32, kind="ExternalInput")
    hnT_out = dt_("hnT_out", [FEAT, NPAD], f16, kind="ExternalOutput")
    tbl = dt_("tbl", [TROWS, 256], f16)
    zeD = dt_("zeD", [FEAT, EPAD], f16)
    heD = dt_("heD", [FEAT, EPAD], f16)
    cc_in = dt_("cc_in", [128, 2], f32)
    cc_out = dt_("cc_out", [128, 2], f32)
    ag_in = dt_("ag_in", [FEAT * NSLICE], f16)
    ag_out = dt_("ag_out", [NCORES * FEAT * NSLICE], f16)

    def allreduce():
        if NCORES == 1:
            nc.sync.dma_start(cc_out[:], cc_in[:])
        else:
            nc.gpsimd.collective_compute(
                "AllReduce", OP.add, replica_groups=RG,
                ins=[cc_in[:].opt()], outs=[cc_out[:].opt()])

    def allgather():
        if NCORES == 1:
            nc.sync.dma_start(ag_out[:], ag_in[:])
        else:
            nc.gpsimd.collective_compute(
                "AllGather", OP.bypass, replica_groups=RG,
                ins=[ag_in[:].opt()], outs=[ag_out[:].opt()])

    with tile.TileContext(nc) as tc:
        with tc.tile_pool(name="persist", bufs=1) as pp:
            hnT = pp.tile([FEAT, NPAD], f16)
            zmg = pp.tile([128, EPAD], f16)
            srcw_s = pp.tile([128, EPAD // 16], mybir.dt.int16)
            dstw_s = pp.tile([128, EPAD // 16], mybir.dt.int16)
            dloc_s = pp.tile([128, NGRP], f32)
            w_ee_s = pp.tile([EDGE_F, FEAT], f16)
            w_emg_s = pp.tile([FEAT, NCONV * 128], f16)
            w_cat_s = pp.tile([FEAT, NCONV * 256], f16)
            gb_e_s = pp.tile([FEAT, 2], f32)
            gb_mg_s = pp.tile([128, NCONV * 2], f32)
            gb_n_s = pp.tile([FEAT, NCONV * 2], f32)
            npad_s = pp.tile([128, 1], f32)
            ident = pp.tile([128, 128], f16)
            iota_row = pp.tile([128, 128], f16)
            sring = pp.tile([128, 2 * NCH + 2], f32)
            st = pp.tile([128, 8], f32)
            sc_m = pp.tile([128, 1], f32)
            sc_t = pp.tile([128, 1], f32)
            eps_t = pp.tile([128, 1], f32)
            nc.vector.memset(eps_t[:], EPS)
            hpad16 = pp.tile([FEAT, 1], f16)
            cpv = pp.tile([128, 1], f32)
            cp2 = pp.tile([128, 1], f32)

            nc.sync.dma_start(hnT[:], hnT0[:])
            nc.sync.dma_start(srcw_s[:], srcw[:])
            nc.sync.dma_start(dstw_s[:], dstw[:])
            nc.sync.dma_start(dloc_s[:], dlocd[:])
            nc.sync.dma_start(w_ee_s[:], w_ee[:])
            nc.sync.dma_start(w_emg_s[:], w_emg[:])
            nc.sync.dma_start(w_cat_s[:], w_cat[:])
            nc.sync.dma_start(gb_e_s[:], gb_e[:])
            nc.sync.dma_start(gb_mg_s[:], gb_mg[:])
            nc.sync.dma_start(gb_n_s[:], gb_n[:])
            nc.sync.dma_start(npad_s[:], npadv[:])
            make_identity(nc, ident[:])
            ii = pp.tile([128, 128], mybir.dt.int32)
            nc.gpsimd.iota(ii[:], pattern=[[1, 128]], base=0,
                           channel_multiplier=0)
            nc.vector.tensor_copy(iota_row[:], ii[:])

            def bn_affine(p, g_ap, b_ap, inv_n):
                """st[:p,0:2] holds (sum, sumsq); writes sc_m/sc_t[:p]."""
                nc.vector.tensor_scalar(out=st[:p, 2:3], in0=st[:p, 0:1],
                                        scalar1=inv_n, scalar2=None,
                                        op0=OP.mult)
                nc.vector.tensor_scalar(out=st[:p, 3:4], in0=st[:p, 1:2],
                                        scalar1=inv_n, scalar2=None,
                                        op0=OP.mult)
                nc.vector.tensor_tensor(out=st[:p, 4:5], in0=st[:p, 2:3],
                                        in1=st[:p, 2:3], op=OP.mult)
                nc.vector.tensor_tensor(out=st[:p, 3:4], in0=st[:p, 3:4],
                                        in1=st[:p, 4:5], op=OP.subtract)
                nc.scalar.activation(st[:p, 3:4], st[:p, 3:4], AF.Sqrt,
                                     bias=eps_t[:p, :])
                nc.vector.reciprocal(st[:p, 3:4], st[:p, 3:4])
                nc.vector.tensor_tensor(out=sc_m[:p, :], in0=st[:p, 3:4],
                                        in1=g_ap, op=OP.mult)
                nc.vector.tensor_tensor(out=st[:p, 5:6], in0=sc_m[:p, :],
                                        in1=st[:p, 2:3], op=OP.mult)
                nc.vector.tensor_tensor(out=sc_t[:p, :], in0=b_ap,
                                        in1=st[:p, 5:6], op=OP.subtract)

            # ---------------- phase E: z = W_ee.T @ efT, stats, silu
            with tc.tile_pool(name="peb", bufs=2) as sb, \
                 tc.tile_pool(name="pep", bufs=2, space="PSUM") as ps:
                for gc in range(NGC):
                    x = sb.tile([EDGE_F, GCH], f16, tag="x")
                    nc.sync.dma_start(x[:], efT[:, gc * GCH:(gc + 1) * GCH])
                    zs = sb.tile([FEAT, GCH], f16, tag="zs")
                    for s in range(GCH // CH):
                        c = gc * (GCH // CH) + s
                        z = ps.tile([FEAT, CH], f32, tag="z")
                        nc.tensor.matmul(
                            z[:], lhsT=w_ee_s[:],
                            rhs=x[:, s * CH:(s + 1) * CH],
                            start=True, stop=True)
                        nc.scalar.activation(zs[:, s * CH:(s + 1) * CH],
                                             z[:], AF.Identity,
                                             accum_out=sring[:FEAT, c:c + 1])
                        sq = sb.tile([FEAT, CH], f16, tag="sq")
                        nc.scalar.activation(
                            sq[:], zs[:, s * CH:(s + 1) * CH], AF.Square,
                            accum_out=sring[:FEAT, NCH + c:NCH + c + 1])
                    nc.sync.dma_start(zeD[:, gc * GCH:(gc + 1) * GCH], zs[:])
            nc.vector.tensor_reduce(out=st[:FEAT, 0:1],
                                    in_=sring[:FEAT, 0:NCH], op=OP.add,
                                    axis=X)
            nc.vector.tensor_reduce(out=st[:FEAT, 1:2],
                                    in_=sring[:FEAT, NCH:2 * NCH], op=OP.add,
                                    axis=X)
            cci = pp.tile([128, 2], f32)
            nc.vector.memset(cci[:], 0.0)
            nc.vector.tensor_copy(cci[:FEAT, :], st[:FEAT, 0:2])
            nc.sync.dma_start(cc_in[:], cci[:])
            allreduce()
            cco = pp.tile([128, 2], f32)
            nc.sync.dma_start(cco[:], cc_out[:])
            nc.vector.tensor_copy(st[:FEAT, 0:2], cco[:FEAT, :])
            bn_affine(FEAT, gb_e_s[:, 0:1], gb_e_s[:, 1:2], 1.0 / E)
            with tc.tile_pool(name="pe2", bufs=2) as sb:
                for gc in range(NGC):
                    zl = sb.tile([FEAT, GCH], f16, tag="zl")
                    nc.sync.dma_start(zl[:], zeD[:, gc * GCH:(gc + 1) * GCH])
                    nc.vector.tensor_scalar(
                        out=zl[:], in0=zl[:], scalar1=sc_m[:FEAT, :],
                        scalar2=sc_t[:FEAT, :], op0=OP.mult, op1=OP.add)
                    sg = sb.tile([FEAT, GCH], f16, tag="sg")
                    nc.scalar.activation(sg[:], zl[:], AF.Sigmoid)
                    nc.vector.tensor_tensor(out=sg[:], in0=zl[:], in1=sg[:],
                                            op=OP.mult)
                    nc.sync.dma_start(heD[:, gc * GCH:(gc + 1) * GCH], sg[:])
            # hpad = silu(t) (z=0 for pad cols)
            hpadf = pp.tile([FEAT, 1], f32)
            nc.scalar.activation(hpadf[:], sc_t[:FEAT, :], AF.Sigmoid)
            nc.vector.tensor_tensor(out=hpadf[:], in0=hpadf[:],
                                    in1=sc_t[:FEAT, :], op=OP.mult)
            nc.vector.tensor_copy(hpad16[:], hpadf[:])

            # ---------------- conv layers
            for l in range(NCONV):
                lsl = slice(l * 128, (l + 1) * 128)
                # tables
                with tc.tile_pool(name=f"tb{l}", bufs=3) as sb, \
                     tc.tile_pool(name=f"tp{l}", bufs=2, space="PSUM") as ps:
                    for c in range(NPAD // 128):
                        t0 = ps.tile([128, 256], f32, tag="t0")
                        nc.tensor.matmul(
                            t0[:], lhsT=hnT[:, c * 128:(c + 1) * 128],
                            rhs=w_cat_s[:, l * 256:(l + 1) * 256],
                            start=True, stop=True)
                        if c % 8 == 0:
                            stg = sb.tile([128, 8, 256], f16, tag="stg")
                        if c % 2 == 0:
                            nc.vector.tensor_copy(stg[:, c % 8, :], t0[:])
                        else:
                            nc.scalar.activation(stg[:, c % 8, :], t0[:],
                                                 AF.Identity)
                        if c % 8 == 7:
                            nc.sync.dma_start(
                                tbl[(c - 7) * 128:(c + 1) * 128, :].rearrange(
                                    "(b p) c -> p b c", b=8), stg[:])
                    if l == 0:
                        zt = sb.tile([16, 256], f16, tag="zt")
                        nc.vector.memset(zt[:], 0.0)
                        nc.sync.dma_start(tbl[NPAD:TROWS, :], zt[:])
                # pad-edge constant
                with tc.tile_pool(name=f"pc{l}", bufs=1, space="PSUM") as ps:
                    cp = ps.tile([128, 1], f32)
                    nc.tensor.matmul(cp[:], lhsT=w_emg_s[:, lsl],
                                     rhs=hpad16[:], start=True, stop=True)
                    nc.vector.tensor_copy(cpv[:], cp[:])
                    nc.vector.tensor_tensor(out=cp2[:], in0=cpv[:],
                                            in1=cpv[:], op=OP.mult)

                # pass1
                with tc.tile_pool(name=f"p1_{l}", bufs=2) as sb, \
                     tc.tile_pool(name=f"g{l}", bufs=2) as gb, \
                     tc.tile_pool(name=f"q{l}", bufs=2, space="PSUM") as ps:
                    for gc in range(NGC):
                        gs = gb.tile([128, 1, GCH], f16, tag="gs")
                        nc.gpsimd.dma_gather(
                            out_ap=gs[:], in_ap=tbl[:, 0:128],
                            idxs_ap=srcw_s[:, gc * (GCH // 16):
                                           (gc + 1) * (GCH // 16)],
                            num_idxs=GCH, num_idxs_reg=GCH, elem_size=128,
                            elem_step=256, transpose=True,
                            single_packet=False)
                        gd = gb.tile([128, 1, GCH], f16, tag="gd")
                        nc.gpsimd.dma_gather(
                            out_ap=gd[:], in_ap=tbl[:, 128:256],
                            idxs_ap=dstw_s[:, gc * (GCH // 16):
                                           (gc + 1) * (GCH // 16)],
                            num_idxs=GCH, num_idxs_reg=GCH, elem_size=128,
                            elem_step=256, transpose=True,
                            single_packet=False)
                        for s in range(GCH // CH):
                            c = gc * (GCH // CH) + s
                            if s % 2 == 0:
                                he = sb.tile([FEAT, 2 * CH], f16, tag="he")
                                nc.sync.dma_start(
                                    he[:], heD[:, c * CH:(c + 2) * CH])
                            z = ps.tile([128, CH], f32, tag="z")
                            nc.tensor.matmul(
                                z[:], lhsT=w_emg_s[:, lsl],
                                rhs=he[:, (s % 2) * CH:(s % 2 + 1) * CH],
                                start=True, stop=False)
                            nc.tensor.matmul(
                                z[:], lhsT=ident[:],
                                rhs=gs[:, 0, s * CH:(s + 1) * CH],
                                start=False, stop=False)
                            nc.tensor.matmul(
                                z[:], lhsT=ident[:],
                                rhs=gd[:, 0, s * CH:(s + 1) * CH],
                                start=False, stop=True)
                            zd = zmg[:, c * CH:(c + 1) * CH]
                            nc.scalar.activation(
                                zd, z[:], AF.Identity,
                                accum_out=sring[:, c:c + 1])
                            sq = ps.tile([128, CH], f32, tag="sq")
                            nc.scalar.activation(
                                sq[:], zd, AF.Square,
                                accum_out=sring[:, NCH + c:NCH + c + 1])
                nc.vector.tensor_reduce(out=st[:, 0:1], in_=sring[:, 0:NCH],
                                        op=OP.add, axis=X)
                nc.vector.tensor_reduce(out=st[:, 1:2],
                                        in_=sring[:, NCH:2 * NCH],
                                        op=OP.add, axis=X)
                nc.vector.tensor_tensor(out=st[:, 2:3], in0=cpv[:],
                                        in1=npad_s[:], op=OP.mult)
                nc.vector.tensor_tensor(out=st[:, 0:1], in0=st[:, 0:1],
                                        in1=st[:, 2:3], op=OP.subtract)
                nc.vector.tensor_tensor(out=st[:, 2:3], in0=cp2[:],
                                        in1=npad_s[:], op=OP.mult)
                nc.vector.tensor_tensor(out=st[:, 1:2], in0=st[:, 1:2],
                                        in1=st[:, 2:3], op=OP.subtract)
                cci2 = pp.tile([128, 2], f32, tag="cci2")
                nc.vector.tensor_copy(cci2[:], st[:, 0:2])
                nc.sync.dma_start(cc_in[:], cci2[:])
                allreduce()
                cco2 = pp.tile([128, 2], f32, tag="cco2")
                nc.sync.dma_start(cco2[:], cc_out[:])
                nc.vector.tensor_copy(st[:, 0:2], cco2[:])
                bn_affine(128, gb_mg_s[:, 2 * l:2 * l + 1],
                          gb_mg_s[:, 2 * l + 1:2 * l + 2], 1.0 / E)

                # pass2
                with tc.tile_pool(name=f"p2_{l}", bufs=2) as sb, \
                     tc.tile_pool(name=f"r2{l}", bufs=2, space="PSUM") as ps, \
                     tc.tile_pool(name=f"a2{l}", bufs=2, space="PSUM") as pa:
                    for gc in range(NGC):
                        zc = zmg[:, gc * GCH:(gc + 1) * GCH]
                        nc.scalar.activation(zc, zc, AF.Sigmoid,
                                             bias=sc_t[:], scale=sc_m[:])
                    for gc in range(NGC):
                        zg = zmg[FEAT:128, gc * GCH:(gc + 1) * GCH]
                        nc.scalar.activation(zg, zg, AF.Ln)
                    aggT = pp.tile([FEAT, NSLICE], f16, tag="aggT")
                    QE = 2048
                    ag = None
                    for blk in range(0, NGRP, 4):
                        nb = min(4, NGRP - blk)
                        e0 = blk * 128
                        if e0 % QE == 0:
                            # move Ln(g) half down to partitions 0-63
                            lb = sb.tile([FEAT, QE], f16, tag="lb")
                            nc.sync.dma_start(
                                lb[:], zmg[FEAT:128, e0:e0 + QE])
                        qo = e0 % QE
                        mt = sb.tile([FEAT, CH], f16, tag="mt")
                        nc.vector.tensor_tensor(
                            out=mt[:, :nb * 128],
                            in0=zmg[0:FEAT, e0:e0 + nb * 128],
                            in1=lb[:, qo:qo + nb * 128], op=OP.mult)
                        mn = ps.tile([128, 4 * FEAT], f16, tag="mn")
                        for j in range(nb):
                            nc.tensor.transpose(
                                mn[:, j * FEAT:(j + 1) * FEAT],
                                mt[:, j * 128:(j + 1) * 128],
                                ident[:FEAT, :FEAT])
                        mns = sb.tile([128, 4 * FEAT], f16, tag="mns")
                        nc.scalar.activation(mns[:, :nb * FEAT],
                                             mn[:, :nb * FEAT], AF.Identity)
                        for j in range(nb):
                            grp = blk + j
                            w, g = grp // GPW, grp % GPW
                            if g == 0:
                                ag = pa.tile([FEAT, 128], f32, tag="ag")
                            oh = sb.tile([128, 128], f16, tag="oh")
                            nc.vector.tensor_scalar(
                                out=oh[:], in0=iota_row[:],
                                scalar1=dloc_s[:, grp:grp + 1],
                                scalar2=-1.0, op0=OP.is_equal, op1=OP.mult)
                            nc.tensor.matmul(
                                ag[:], lhsT=mns[:, j * FEAT:(j + 1) * FEAT],
                                rhs=oh[:], start=(g == 0),
                                stop=(g == GPW - 1))
                            if g == GPW - 1:
                                nc.scalar.activation(
                                    aggT[:, w * 128:(w + 1) * 128], ag[:],
                                    AF.Identity)
                    nc.sync.dma_start(
                        ag_in[:].rearrange("(a b) -> a b", a=FEAT), aggT[:])
                allgather()
                # agg stats + h_n update
                HS = NSLICE // 2
                NPC = 2 * NCORES  # pieces of [FEAT, HS]
                agr = ag_out[:].rearrange("(c f hh h) -> c f hh h",
                                          c=NCORES, f=FEAT, hh=2)
                with tc.tile_pool(name=f"u{l}", bufs=2) as sb:
                    for k in range(NPC):
                        t = sb.tile([FEAT, HS], f16, tag="agld")
                        nc.sync.dma_start(t[:], agr[k // 2][:, k % 2, :])
                        d1 = sb.tile([FEAT, HS], f16, tag="dsink")
                        nc.scalar.activation(
                            d1[:], t[:], AF.Identity,
                            accum_out=sring[:FEAT, k:k + 1])
                        d2 = sb.tile([FEAT, HS], f16, tag="dsink")
                        nc.scalar.activation(
                            d2[:], t[:], AF.Square,
                            accum_out=sring[:FEAT, NPC + k:NPC + k + 1])
                    nc.vector.tensor_reduce(
                        out=st[:FEAT, 0:1], in_=sring[:FEAT, 0:NPC],
                        op=OP.add, axis=X)
                    nc.vector.tensor_reduce(
                        out=st[:FEAT, 1:2], in_=sring[:FEAT, NPC:2 * NPC],
                        op=OP.add, axis=X)
                    bn_affine(FEAT, gb_n_s[:, 2 * l:2 * l + 1],
                              gb_n_s[:, 2 * l + 1:2 * l + 2], 1.0 / N)
                    for k in range(NPC):
                        t = sb.tile([FEAT, HS], f16, tag="agld")
                        nc.sync.dma_start(t[:], agr[k // 2][:, k % 2, :])
                        tmp = sb.tile([FEAT, HS], f16, tag="tmp")
                        nc.vector.tensor_scalar(
                            out=tmp[:], in0=t[:], scalar1=sc_m[:FEAT, :],
                            scalar2=sc_t[:FEAT, :], op0=OP.mult, op1=OP.add)
                        hsl = hnT[:, k * HS:(k + 1) * HS]
                        nc.vector.tensor_tensor(out=tmp[:], in0=tmp[:],
                                                in1=hsl, op=OP.add)
                        nc.scalar.activation(hsl, tmp[:], AF.Sigmoid)
            nc.sync.dma_start(hnT_out[:], hnT[:])
    nc.compile()
    return nc


# ------------------------------------------------------------------- kernel
def _silu(x):
    return x / (1.0 + np.exp(-x))


def _bn(x, g, b):
    return g * (x - x.mean(0)) / np.sqrt(x.var(0) + EPS) + b


def make_in_maps(inputs, prep):
    """Host-side marshaling: returns (in_maps, host_ctx)."""
    f32 = lambda k: np.asarray(inputs[k], np.float32)
    node_feats = f32("node_feats")
    edge_feats = f32("edge_feats")
    EPAD = prep["EPAD"]

    h_n0 = _silu(_bn(node_feats @ f32("W_ne"), f32("g_ne"), f32("be_ne")))
    hnT0 = np.zeros((FEAT, NPAD), np.float16)
    hnT0[:, :N] = h_n0.T.astype(np.float16)

    Wm, Wg = f32("Wm"), f32("Wg")
    w_ee = f32("W_ee").astype(np.float16)
    w_emg = np.concatenate(
        [np.concatenate([Wm[l][2 * FEAT:], Wg[l][2 * FEAT:]], 1)
         for l in range(NCONV)], 1).astype(np.float16)
    w_cat = np.concatenate(
        [np.concatenate([Wm[l][:FEAT], Wg[l][:FEAT],
                         Wm[l][FEAT:2 * FEAT], Wg[l][FEAT:2 * FEAT]], 1)
         for l in range(NCONV)], 1).astype(np.float16)
    gb_e = np.ascontiguousarray(
        np.stack([f32("g_ee"), f32("be_ee")], 1).astype(np.float32))
    gb_mg = np.zeros((128, NCONV * 2), np.float32)
    gb_n = np.zeros((FEAT, NCONV * 2), np.float32)
    for l in range(NCONV):
        gb_mg[:FEAT, 2 * l] = f32("gm")[l]
        gb_mg[FEAT:, 2 * l] = -f32("gg")[l]
        gb_mg[:FEAT, 2 * l + 1] = f32("bem")[l]
        gb_mg[FEAT:, 2 * l + 1] = -f32("beg")[l]
        gb_n[:, 2 * l] = f32("gn")[l]
        gb_n[:, 2 * l + 1] = f32("ben")[l]

    in_maps = []
    for k in range(NCORES):
        efT = np.zeros((EDGE_F, EPAD), np.float16)
        valid = prep["eperm"][k] >= 0
        efT[:, valid] = edge_feats[prep["eperm"][k][valid]].T.astype(
            np.float16)
        npadv = np.full((128, 1), float(EPAD - valid.sum()), np.float32)
        in_maps.append(dict(
            efT=efT, hnT0=hnT0, srcw=prep["src_w"][k], dstw=prep["dst_w"][k],
            dlocd=np.ascontiguousarray(prep["dloc"][k]), w_ee=w_ee,
            w_emg=w_emg, w_cat=w_cat, gb_e=gb_e, gb_mg=gb_mg, gb_n=gb_n,
            npadv=npadv))
    return in_maps


def head(inputs, hnT):
    f32 = lambda k: np.asarray(inputs[k], np.float32)
    n2g = np.asarray(inputs["node2graph"], np.int64)
    h_n = hnT[:, :N].T.astype(np.float32)
    sums = np.zeros((G, FEAT), np.float32)
    np.add.at(sums, n2g, h_n)
    cnt = np.bincount(n2g, minlength=G).astype(np.float32)[:, None]
    pooled = sums / np.maximum(cnt, 1.0)
    h = _silu(_bn(pooled @ f32("W_fc") + f32("b_fc"), f32("g_fc"),
                  f32("be_fc")))
    return (h @ f32("W_out") + f32("b_out")).astype(np.float32)


def kernel(**inputs):
    import time as _time
    from concourse.bass_utils import run_bass_kernel_spmd

    src = np.asarray(inputs["src"], np.int64)
    dst = np.asarray(inputs["dst"], np.int64)
    prep = _host_prep(src, dst)
    key = ("nc", prep["EPAD"], prep["GPW"])
    if key not in _cache:
        _cache[key] = _build(prep["EPAD"], prep["GPW"])
        try:
            from concourse.timeline_sim import TimelineSim
            globals()["LAST_EXEC_NS"] = int(
                TimelineSim(_cache[key], no_exec=True).simulate())
        except Exception:
            pass
    nc = _cache[key]
    in_maps = make_in_maps(inputs, prep)
    t0 = _time.time()
    res = run_bass_kernel_spmd(nc, in_maps, core_ids=list(range(NCORES)))
    globals()["LAST_WALL_S"] = _time.time() - t0
    hnT = res.results[0]["hnT_out"].astype(np.float32)
    return head(inputs, hnT)


# revision 4
# speedup vs baseline: 1.3104x; 1.0170x over previous
"""CGCNN forward on 8 Trainium2 NeuronCores — conv layers fully on-device.

Layout: transposed (features on partitions, entities on free), fp16 data.
Edges sorted by dst; core k owns nodes [k*NPAD/8, (k+1)*NPAD/8) and all
edges into them, grouped 128-per-128-node-window (uniform GPW groups per
window for SPMD). h_n replicated; per-layer: AllReduce of BN stats (1KB) +
AllGather of agg slices. Node embedding and graph pooling/head on host.
"""
import sys
sys.path.insert(0, "/opt/trn_rl_repo")
import numpy as np

EPS = 1e-5
NODE_F, EDGE_F, FEAT, NCONV = 92, 41, 64, 3

# problem sizes (overridable for mini tests)
N, E, G = 25000, 400000, 128
NCORES = 8
NPAD = 25600

_cache = {}


def _derived():
    NW = NPAD // 128 // NCORES
    NSLICE = NPAD // NCORES
    ZROW = NPAD
    TROWS = NPAD + 16
    return NW, NSLICE, ZROW, TROWS


# ----------------------------------------------------------------- host prep
def _host_prep(src, dst):
    NW, NSLICE, ZROW, TROWS = _derived()
    order = np.argsort(dst, kind="stable")
    dsts = dst[order]
    srcs = src[order]
    nwin = NPAD // 128
    win = dsts // 128
    wcnt = np.bincount(win, minlength=nwin)
    GPW = int(np.max((wcnt + 127) // 128))
    NGRP = NW * GPW
    EPAD = ((NGRP * 128 + 2047) // 2048) * 2048
    wstart = np.concatenate([[0], np.cumsum(wcnt)])
    src_idx = np.full((NCORES, EPAD), ZROW, np.int16)
    dst_idx = np.full((NCORES, EPAD), ZROW, np.int16)
    dloc = np.full((NCORES, NGRP * 128), -1.0, np.float32)
    eperm = np.full((NCORES, EPAD), -1, np.int64)
    for k in range(NCORES):
        for w in range(NW):
            gw = k * NW + w
            a, b = wstart[gw], wstart[gw + 1]
            ne = b - a
            base = w * GPW * 128
            src_idx[k, base:base + ne] = srcs[a:b].astype(np.int16)
            dst_idx[k, base:base + ne] = dsts[a:b].astype(np.int16)
            dloc[k, base:base + ne] = (dsts[a:b] - gw * 128).astype(np.float32)
            eperm[k, base:base + ne] = order[a:b]

    def wrap16(idx2d):
        out = np.zeros((NCORES, 128, EPAD // 16), np.int16)
        for k in range(NCORES):
            blk = idx2d[k].reshape(EPAD // 16, 16).T
            for c in range(8):
                out[k, c * 16:(c + 1) * 16, :] = blk
        return out

    # dloc per-partition layout: [128 edge-in-group, NGRP]
    dloc_pp = dloc.reshape(NCORES, NGRP, 128).transpose(0, 2, 1).copy()
    return dict(GPW=GPW, EPAD=EPAD, NGRP=NGRP, src_w=wrap16(src_idx),
                dst_w=wrap16(dst_idx), dloc=dloc_pp, eperm=eperm)


# ------------------------------------------------------------ module builder
def _build(EPAD, GPW):
    import concourse.bacc as bacc
    import concourse.mybir as mybir
    import concourse.tile as tile
    from concourse.masks import make_identity

    NW, NSLICE, ZROW, TROWS = _derived()
    f16, f32 = mybir.dt.float16, mybir.dt.float32
    AF = mybir.ActivationFunctionType
    OP = mybir.AluOpType
    X = mybir.AxisListType.X
    NGRP = NW * GPW
    CH = 512
    NCH = EPAD // CH
    GCH = 2048
    NGC = EPAD // GCH
    assert EPAD % GCH == 0 and EPAD % CH == 0 and EPAD >= NGRP * 128
    RG = [[i for i in range(NCORES)]]

    nc = bacc.Bacc("TRN2", target_bir_lowering=False, debug=False,
                   num_devices=NCORES)
    dt_ = nc.dram_tensor
    efT = dt_("efT", [EDGE_F, EPAD], f16, kind="ExternalInput")
    hnT0 = dt_("hnT0", [FEAT, NPAD], f16, kind="ExternalInput")
    srcw = dt_("srcw", [128, EPAD // 16], mybir.dt.int16, kind="ExternalInput")
    dstw = dt_("dstw", [128, EPAD // 16], mybir.dt.int16, kind="ExternalInput")
    dlocd = dt_("dlocd", [128, NGRP], f32, kind="ExternalInput")
    w_ee = dt_("w_ee", [EDGE_F, FEAT], f16, kind="ExternalInput")
    w_emg = dt_("w_emg", [FEAT, NCONV * 128], f16, kind="ExternalInput")
    w_cat = dt_("w_cat", [FEAT, NCONV * 256], f16, kind="ExternalInput")
    gb_e = dt_("gb_e", [FEAT, 2], f32, kind="ExternalInput")
    gb_mg = dt_("gb_mg", [128, NCONV * 2], f32, kind="ExternalInput")
    gb_n = dt_("gb_n", [FEAT, NCONV * 2], f32, kind="ExternalInput")
    npadv = dt_("npadv", [128, 1], f32, kind="ExternalInput")
    hnT_out = dt_("hnT_out", [FEAT, NPAD], f16, kind="ExternalOutput")
    tbl = dt_("tbl", [TROWS, 256], f16)
    zeD = dt_("zeD", [FEAT, EPAD], f16)
    heD = dt_("heD", [FEAT, EPAD], f16)
    cc_in = dt_("cc_in", [128, 2], f32)
    cc_out = dt_("cc_out", [128, 2], f32)
    ag_in = dt_("ag_in", [FEAT * NSLICE], f16)
    ag_out = dt_("ag_out", [NCORES * FEAT * NSLICE], f16)

    def allreduce():
        if NCORES == 1:
            nc.sync.dma_start(cc_out[:], cc_in[:])
        else:
            nc.gpsimd.collective_compute(
                "AllReduce", OP.add, replica_groups=RG,
                ins=[cc_in[:].opt()], outs=[cc_out[:].opt()])

    def allgather():
        if NCORES == 1:
            nc.sync.dma_start(ag_out[:], ag_in[:])
        else:
            nc.gpsimd.collective_compute(
                "AllGather", OP.bypass, replica_groups=RG,
                ins=[ag_in[:].opt()], outs=[ag_out[:].opt()])

    with tile.TileContext(nc) as tc:
        with tc.tile_pool(name="persist", bufs=1) as pp:
            hnT = pp.tile([FEAT, NPAD], f16)
            zmg = pp.tile([128, EPAD], f16)
            srcw_s = pp.tile([128, EPAD // 16], mybir.dt.int16)
            dstw_s = pp.tile([128, EPAD // 16], mybir.dt.int16)
            dloc_s = pp.tile([128, NGRP], f32)
            w_ee_s = pp.tile([EDGE_F, FEAT], f16)
            w_emg_s = pp.tile([FEAT, NCONV * 128], f16)
            w_cat_s = pp.tile([FEAT, NCONV * 256], f16)
            gb_e_s = pp.tile([FEAT, 2], f32)
            gb_mg_s = pp.tile([128, NCONV * 2], f32)
            gb_n_s = pp.tile([FEAT, NCONV * 2], f32)
            npad_s = pp.tile([128, 1], f32)
            ident = pp.tile([128, 128], f16)
            iota_row = pp.tile([128, 128], f16)
            sring = pp.tile([128, 2 * NCH + 2], f32)
            st = pp.tile([128, 8], f32)
            sc_m = pp.tile([128, 1], f32)
            sc_t = pp.tile([128, 1], f32)
            eps_t = pp.tile([128, 1], f32)
            nc.vector.memset(eps_t[:], EPS)
            hpad16 = pp.tile([FEAT, 1], f16)
            cpv = pp.tile([128, 1], f32)
            cp2 = pp.tile([128, 1], f32)

            nc.sync.dma_start(hnT[:], hnT0[:])
            nc.sync.dma_start(srcw_s[:], srcw[:])
            nc.sync.dma_start(dstw_s[:], dstw[:])
            nc.sync.dma_start(dloc_s[:], dlocd[:])
            nc.sync.dma_start(w_ee_s[:], w_ee[:])
            nc.sync.dma_start(w_emg_s[:], w_emg[:])
            nc.sync.dma_start(w_cat_s[:], w_cat[:])
            nc.sync.dma_start(gb_e_s[:], gb_e[:])
            nc.sync.dma_start(gb_mg_s[:], gb_mg[:])
            nc.sync.dma_start(gb_n_s[:], gb_n[:])
            nc.sync.dma_start(npad_s[:], npadv[:])
            make_identity(nc, ident[:])
            ii = pp.tile([128, 128], mybir.dt.int32)
            nc.gpsimd.iota(ii[:], pattern=[[1, 128]], base=0,
                           channel_multiplier=0)
            nc.vector.tensor_copy(iota_row[:], ii[:])

            def bn_affine(p, g_ap, b_ap, inv_n):
                """st[:p,0:2] holds (sum, sumsq); writes sc_m/sc_t[:p]."""
                nc.vector.tensor_scalar(out=st[:p, 2:3], in0=st[:p, 0:1],
                                        scalar1=inv_n, scalar2=None,
                                        op0=OP.mult)
                nc.vector.tensor_scalar(out=st[:p, 3:4], in0=st[:p, 1:2],
                                        scalar1=inv_n, scalar2=None,
                                        op0=OP.mult)
                nc.vector.tensor_tensor(out=st[:p, 4:5], in0=st[:p, 2:3],
                                        in1=st[:p, 2:3], op=OP.mult)
                nc.vector.tensor_tensor(out=st[:p, 3:4], in0=st[:p, 3:4],
                                        in1=st[:p, 4:5], op=OP.subtract)
                nc.scalar.activation(st[:p, 3:4], st[:p, 3:4], AF.Sqrt,
                                     bias=eps_t[:p, :])
                nc.vector.reciprocal(st[:p, 3:4], st[:p, 3:4])
                nc.vector.tensor_tensor(out=sc_m[:p, :], in0=st[:p, 3:4],
                                        in1=g_ap, op=OP.mult)
                nc.vector.tensor_tensor(out=st[:p, 5:6], in0=sc_m[:p, :],
                                        in1=st[:p, 2:3], op=OP.mult)
                nc.vector.tensor_tensor(out=sc_t[:p, :], in0=b_ap,
                                        in1=st[:p, 5:6], op=OP.subtract)

            # ---------------- phase E: z = W_ee.T @ efT, stats, silu
            with tc.tile_pool(name="peb", bufs=2) as sb, \
                 tc.tile_pool(name="pep", bufs=2, space="PSUM") as ps:
                for gc in range(NGC):
                    x = sb.tile([EDGE_F, GCH], f16, tag="x")
                    nc.sync.dma_start(x[:], efT[:, gc * GCH:(gc + 1) * GCH])
                    zs = sb.tile([FEAT, GCH], f16, tag="zs")
                    for s in range(GCH // CH):
                        c = gc * (GCH // CH) + s
                        z = ps.tile([FEAT, CH], f32, tag="z")
                        nc.tensor.matmul(
                            z[:], lhsT=w_ee_s[:],
                            rhs=x[:, s * CH:(s + 1) * CH],
                            start=True, stop=True)
                        nc.scalar.activation(zs[:, s * CH:(s + 1) * CH],
                                             z[:], AF.Identity,
                                             accum_out=sring[:FEAT, c:c + 1])
                        sq = sb.tile([FEAT, CH], f16, tag="sq")
                        nc.scalar.activation(
                            sq[:], zs[:, s * CH:(s + 1) * CH], AF.Square,
                            accum_out=sring[:FEAT, NCH + c:NCH + c + 1])
                    nc.sync.dma_start(zeD[:, gc * GCH:(gc + 1) * GCH], zs[:])
            nc.vector.tensor_reduce(out=st[:FEAT, 0:1],
                                    in_=sring[:FEAT, 0:NCH], op=OP.add,
                                    axis=X)
            nc.vector.tensor_reduce(out=st[:FEAT, 1:2],
                                    in_=sring[:FEAT, NCH:2 * NCH], op=OP.add,
                                    axis=X)
            cci = pp.tile([128, 2], f32)
            nc.vector.memset(cci[:], 0.0)
            nc.vector.tensor_copy(cci[:FEAT, :], st[:FEAT, 0:2])
            nc.sync.dma_start(cc_in[:], cci[:])
            allreduce()
            cco = pp.tile([128, 2], f32)
            nc.sync.dma_start(cco[:], cc_out[:])
            nc.vector.tensor_copy(st[:FEAT, 0:2], cco[:FEAT, :])
            bn_affine(FEAT, gb_e_s[:, 0:1], gb_e_s[:, 1:2], 1.0 / E)
            with tc.tile_pool(name="pe2", bufs=2) as sb:
                for gc in range(NGC):
                    zl = sb.tile([FEAT, GCH], f16, tag="zl")
                    nc.sync.dma_start(zl[:], zeD[:, gc * GCH:(gc + 1) * GCH])
                    nc.vector.tensor_scalar(
                        out=zl[:], in0=zl[:], scalar1=sc_m[:FEAT, :],
                        scalar2=sc_t[:FEAT, :], op0=OP.mult, op1=OP.add)
                    sg = sb.tile([FEAT, GCH], f16, tag="sg")
                    nc.scalar.activation(sg[:], zl[:], AF.Sigmoid)
                    nc.vector.tensor_tensor(out=sg[:], in0=zl[:], in1=sg[:],
                                            op=OP.mult)
                    nc.sync.dma_start(heD[:, gc * GCH:(gc + 1) * GCH], sg[:])
            # hpad = silu(t) (z=0 for pad cols)
            hpadf = pp.tile([FEAT, 1], f32)
            nc.scalar.activation(hpadf[:], sc_t[:FEAT, :], AF.Sigmoid)
            nc.vector.tensor_tensor(out=hpadf[:], in0=hpadf[:],
                                    in1=sc_t[:FEAT, :], op=OP.mult)
            nc.vector.tensor_copy(hpad16[:], hpadf[:])

            # ---------------- conv layers
            for l in range(NCONV):
                lsl = slice(l * 128, (l + 1) * 128)
                # tables
                with tc.tile_pool(name=f"tb{l}", bufs=3) as sb, \
                     tc.tile_pool(name=f"tp{l}", bufs=2, space="PSUM") as ps:
                    for c in range(NPAD // 128):
                        t0 = ps.tile([128, 256], f32, tag="t0")
                        nc.tensor.matmul(
                            t0[:], lhsT=hnT[:, c * 128:(c + 1) * 128],
                            rhs=w_cat_s[:, l * 256:(l + 1) * 256],
                            start=True, stop=True)
                        if c % 8 == 0:
                            stg = sb.tile([128, 8, 256], f16, tag="stg")
                        if c % 2 == 0:
                            nc.vector.tensor_copy(stg[:, c % 8, :], t0[:])
                        else:
                            nc.scalar.activation(stg[:, c % 8, :], t0[:],
                                                 AF.Identity)
                        if c % 8 == 7:
                            nc.sync.dma_start(
                                tbl[(c - 7) * 128:(c + 1) * 128, :].rearrange(
                                    "(b p) c -> p b c", b=8), stg[:])
                    if l == 0:
                        zt = sb.tile([16, 256], f16, tag="zt")
                        nc.vector.memset(zt[:], 0.0)
                        nc.sync.dma_start(tbl[NPAD:TROWS, :], zt[:])
                # pad-edge constant
                with tc.tile_pool(name=f"pc{l}", bufs=1, space="PSUM") as ps:
                    cp = ps.tile([128, 1], f32)
                    nc.tensor.matmul(cp[:], lhsT=w_emg_s[:, lsl],
                                     rhs=hpad16[:], start=True, stop=True)
                    nc.vector.tensor_copy(cpv[:], cp[:])
                    nc.vector.tensor_tensor(out=cp2[:], in0=cpv[:],
                                            in1=cpv[:], op=OP.mult)

                # pass1
                with tc.tile_pool(name=f"p1_{l}", bufs=2) as sb, \
                     tc.tile_pool(name=f"g{l}", bufs=2) as gb, \
                     tc.tile_pool(name=f"q{l}", bufs=2, space="PSUM") as ps:
                    for gc in range(NGC):
                        gs = gb.tile([128, 1, GCH], f16, tag="gs")
                        nc.gpsimd.dma_gather(
                            out_ap=gs[:], in_ap=tbl[:, 0:128],
                            idxs_ap=srcw_s[:, gc * (GCH // 16):
                                           (gc + 1) * (GCH // 16)],
                            num_idxs=GCH, num_idxs_reg=GCH, elem_size=128,
                            elem_step=256, transpose=True,
                            single_packet=False)
                        gd = gb.tile([128, 1, GCH], f16, tag="gd")
                        nc.gpsimd.dma_gather(
                            out_ap=gd[:], in_ap=tbl[:, 128:256],
                            idxs_ap=dstw_s[:, gc * (GCH // 16):
                                           (gc + 1) * (GCH // 16)],
                            num_idxs=GCH, num_idxs_reg=GCH, elem_size=128,
                            elem_step=256, transpose=True,
                            single_packet=False)
                        for s in range(GCH // CH):
                            c = gc * (GCH // CH) + s
                            if s % 2 == 0:
                                he = sb.tile([FEAT, 2 * CH], f16, tag="he")
                                nc.sync.dma_start(
                                    he[:], heD[:, c * CH:(c + 2) * CH])
                            z = ps.tile([128, CH], f32, tag="z")
                            nc.tensor.matmul(
                                z[:], lhsT=w_emg_s[:, lsl],
                                rhs=he[:, (s % 2) * CH:(s % 2 + 1) * CH],
                                start=True, stop=False)
                            nc.tensor.matmul(
                                z[:], lhsT=ident[:],
                                rhs=gs[:, 0, s * CH:(s + 1) * CH],
                                start=False, stop=False)
                            nc.tensor.matmul(
                                z[:], lhsT=ident[:],
                                rhs=gd[:, 0, s * CH:(s + 1) * CH],
                                start=False, stop=True)
                            zd = zmg[:, c * CH:(c + 1) * CH]
                            nc.vector.tensor_scalar(
                                out=zd, in0=z[:], scalar1=1.0, scalar2=0.0,
                                op0=OP.mult, op1=OP.add,
                                accum_out=sring[:, c:c + 1])
                            sq = ps.tile([128, CH], f32, tag="sq")
                            nc.scalar.activation(
                                sq[:], zd, AF.Square,
                                accum_out=sring[:, NCH + c:NCH + c + 1])
                nc.vector.tensor_reduce(out=st[:, 0:1], in_=sring[:, 0:NCH],
                                        op=OP.add, axis=X)
                nc.vector.tensor_reduce(out=st[:, 1:2],
                                        in_=sring[:, NCH:2 * NCH],
                                        op=OP.add, axis=X)
                nc.vector.tensor_tensor(out=st[:, 2:3], in0=cpv[:],
                                        in1=npad_s[:], op=OP.mult)
                nc.vector.tensor_tensor(out=st[:, 0:1], in0=st[:, 0:1],
                                        in1=st[:, 2:3], op=OP.subtract)
                nc.vector.tensor_tensor(out=st[:, 2:3], in0=cp2[:],
                                        in1=npad_s[:], op=OP.mult)
                nc.vector.tensor_tensor(out=st[:, 1:2], in0=st[:, 1:2],
                                        in1=st[:, 2:3], op=OP.subtract)
                cci2 = pp.tile([128, 2], f32, tag="cci2")
                nc.vector.tensor_copy(cci2[:], st[:, 0:2])
                nc.sync.dma_start(cc_in[:], cci2[:])
                allreduce()
                cco2 = pp.tile([128, 2], f32, tag="cco2")
                nc.sync.dma_start(cco2[:], cc_out[:])
                nc.vector.tensor_copy(st[:, 0:2], cco2[:])
                bn_affine(128, gb_mg_s[:, 2 * l:2 * l + 1],
                          gb_mg_s[:, 2 * l + 1:2 * l + 2], 1.0 / E)

                # pass2
                with tc.tile_pool(name=f"p2_{l}", bufs=2) as sb, \
                     tc.tile_pool(name=f"r2{l}", bufs=2, space="PSUM") as ps, \
                     tc.tile_pool(name=f"a2{l}", bufs=2, space="PSUM") as pa:
                    for gc in range(NGC):
                        zc = zmg[:, gc * GCH:(gc + 1) * GCH]
                        nc.scalar.activation(zc, zc, AF.Sigmoid,
                                             bias=sc_t[:], scale=sc_m[:])
                    for gc in range(NGC):
                        zg = zmg[FEAT:128, gc * GCH:(gc + 1) * GCH]
                        nc.scalar.activation(zg, zg, AF.Ln)
                    aggT = pp.tile([FEAT, NSLICE], f16, tag="aggT")
                    QE = 2048
                    ag = None
                    for blk in range(0, NGRP, 4):
                        nb = min(4, NGRP - blk)
                        e0 = blk * 128
                        if e0 % QE == 0:
                            # move Ln(g) half down to partitions 0-63
                            lb = sb.tile([FEAT, QE], f16, tag="lb")
                            nc.sync.dma_start(
                                lb[:], zmg[FEAT:128, e0:e0 + QE])
                        qo = e0 % QE
                        mt = sb.tile([FEAT, CH], f16, tag="mt")
                        nc.vector.tensor_tensor(
                            out=mt[:, :nb * 128],
                            in0=zmg[0:FEAT, e0:e0 + nb * 128],
                            in1=lb[:, qo:qo + nb * 128], op=OP.mult)
                        mn = ps.tile([128, 4 * FEAT], f16, tag="mn")
                        for j in range(nb):
                            nc.tensor.transpose(
                                mn[:, j * FEAT:(j + 1) * FEAT],
                                mt[:, j * 128:(j + 1) * 128],
                                ident[:FEAT, :FEAT])
                        mns = sb.tile([128, 4 * FEAT], f16, tag="mns")
                        nc.vector.tensor_copy(mns[:, :nb * FEAT],
                                              mn[:, :nb * FEAT])
                        for j in range(nb):
                            grp = blk + j
                            w, g = grp // GPW, grp % GPW
                            if g == 0:
                                ag = pa.tile([FEAT, 128], f32, tag="ag")
                            oh = sb.tile([128, 128], f16, tag="oh")
                            nc.vector.tensor_scalar(
                                out=oh[:], in0=iota_row[:],
                                scalar1=dloc_s[:, grp:grp + 1],
                                scalar2=-1.0, op0=OP.is_equal, op1=OP.mult)
                            nc.tensor.matmul(
                                ag[:], lhsT=mns[:, j * FEAT:(j + 1) * FEAT],
                                rhs=oh[:], start=(g == 0),
                                stop=(g == GPW - 1))
                            if g == GPW - 1:
                                nc.scalar.activation(
                                    aggT[:, w * 128:(w + 1) * 128], ag[:],
                                    AF.Identity)
                    nc.sync.dma_start(
                        ag_in[:].rearrange("(a b) -> a b", a=FEAT), aggT[:])
                allgather()
                # agg stats + h_n update
                HS = NSLICE // 2
                NPC = 2 * NCORES  # pieces of [FEAT, HS]
                agr = ag_out[:].rearrange("(c f hh h) -> c f hh h",
                                          c=NCORES, f=FEAT, hh=2)
                with tc.tile_pool(name=f"u{l}", bufs=2) as sb:
                    for k in range(NPC):
                        t = sb.tile([FEAT, HS], f16, tag="agld")
                        nc.sync.dma_start(t[:], agr[k // 2][:, k % 2, :])
                        d1 = sb.tile([FEAT, HS], f16, tag="dsink")
                        nc.scalar.activation(
                            d1[:], t[:], AF.Identity,
                            accum_out=sring[:FEAT, k:k + 1])
                        d2 = sb.tile([FEAT, HS], f16, tag="dsink")
                        nc.scalar.activation(
                            d2[:], t[:], AF.Square,
                            accum_out=sring[:FEAT, NPC + k:NPC + k + 1])
                    nc.vector.tensor_reduce(
                        out=st[:FEAT, 0:1], in_=sring[:FEAT, 0:NPC],
                        op=OP.add, axis=X)
                    nc.vector.tensor_reduce(
                        out=st[:FEAT, 1:2], in_=sring[:FEAT, NPC:2 * NPC],
                        op=OP.add, axis=X)
                    bn_affine(FEAT, gb_n_s[:, 2 * l:2 * l + 1],
                              gb_n_s[:, 2 * l + 1:2 * l + 2], 1.0 / N)
                    for k in range(NPC):
                        t = sb.tile([FEAT, HS], f16, tag="agld")
                        nc.sync.dma_start(t[:], agr[k // 2][:, k % 2, :])
                        tmp = sb.tile([FEAT, HS], f16, tag="tmp")
                        nc.vector.tensor_scalar(
                            out=tmp[:], in0=t[:], scalar1=sc_m[:FEAT, :],
                            scalar2=sc_t[:FEAT, :], op0=OP.mult, op1=OP.add)
                        hsl = hnT[:, k * HS:(k + 1) * HS]
                        nc.vector.tensor_tensor(out=tmp[:], in0=tmp[:],
                                                in1=hsl, op=OP.add)
                        nc.scalar.activation(hsl, tmp[:], AF.Sigmoid)
            nc.sync.dma_start(hnT_out[:], hnT[:])
    nc.compile()
    return nc


# ------------------------------------------------------------------- kernel
def _silu(x):
    return x / (1.0 + np.exp(-x))


def _bn(x, g, b):
    return g * (x - x.mean(0)) / np.sqrt(x.var(0) + EPS) + b


def make_in_maps(inputs, prep):
    """Host-side marshaling: returns (in_maps, host_ctx)."""
    f32 = lambda k: np.asarray(inputs[k], np.float32)
    node_feats = f32("node_feats")
    edge_feats = f32("edge_feats")
    EPAD = prep["EPAD"]

    h_n0 = _silu(_bn(node_feats @ f32("W_ne"), f32("g_ne"), f32("be_ne")))
    hnT0 = np.zeros((FEAT, NPAD), np.float16)
    hnT0[:, :N] = h_n0.T.astype(np.float16)

    Wm, Wg = f32("Wm"), f32("Wg")
    w_ee = f32("W_ee").astype(np.float16)
    w_emg = np.concatenate(
        [np.concatenate([Wm[l][2 * FEAT:], Wg[l][2 * FEAT:]], 1)
         for l in range(NCONV)], 1).astype(np.float16)
    w_cat = np.concatenate(
        [np.concatenate([Wm[l][:FEAT], Wg[l][:FEAT],
                         Wm[l][FEAT:2 * FEAT], Wg[l][FEAT:2 * FEAT]], 1)
         for l in range(NCONV)], 1).astype(np.float16)
    gb_e = np.ascontiguousarray(
        np.stack([f32("g_ee"), f32("be_ee")], 1).astype(np.float32))
    gb_mg = np.zeros((128, NCONV * 2), np.float32)
    gb_n = np.zeros((FEAT, NCONV * 2), np.float32)
    for l in range(NCONV):
        gb_mg[:FEAT, 2 * l] = f32("gm")[l]
        gb_mg[FEAT:, 2 * l] = -f32("gg")[l]
        gb_mg[:FEAT, 2 * l + 1] = f32("bem")[l]
        gb_mg[FEAT:, 2 * l + 1] = -f32("beg")[l]
        gb_n[:, 2 * l] = f32("gn")[l]
        gb_n[:, 2 * l + 1] = f32("ben")[l]

    in_maps = []
    for k in range(NCORES):
        efT = np.zeros((EDGE_F, EPAD), np.float16)
        valid = prep["eperm"][k] >= 0
        efT[:, valid] = edge_feats[prep["eperm"][k][valid]].T.astype(
            np.float16)
        npadv = np.full((128, 1), float(EPAD - valid.sum()), np.float32)
        in_maps.append(dict(
            efT=efT, hnT0=hnT0, srcw=prep["src_w"][k], dstw=prep["dst_w"][k],
            dlocd=np.ascontiguousarray(prep["dloc"][k]), w_ee=w_ee,
            w_emg=w_emg, w_cat=w_cat, gb_e=gb_e, gb_mg=gb_mg, gb_n=gb_n,
            npadv=npadv))
    return in_maps


def head(inputs, hnT):
    f32 = lambda k: np.asarray(inputs[k], np.float32)
    n2g = np.asarray(inputs["node2graph"], np.int64)
    h_n = hnT[:, :N].T.astype(np.float32)
    sums = np.zeros((G, FEAT), np.float32)
    np.add.at(sums, n2g, h_n)
    cnt = np.bincount(n2g, minlength=G).astype(np.float32)[:, None]
    pooled = sums / np.maximum(cnt, 1.0)
    h = _silu(_bn(pooled @ f32("W_fc") + f32("b_fc"), f32("g_fc"),
                  f32("be_fc")))
    return (h @ f32("W_out") + f32("b_out")).astype(np.float32)


def kernel(**inputs):
    import time as _time
    from concourse.bass_utils import run_bass_kernel_spmd

    src = np.asarray(inputs["src"], np.int64)
    dst = np.asarray(inputs["dst"], np.int64)
    prep = _host_prep(src, dst)
    key = ("nc", prep["EPAD"], prep["GPW"])
    if key not in _cache:
        _cache[key] = _build(prep["EPAD"], prep["GPW"])
        try:
            from concourse.timeline_sim import TimelineSim
            globals()["LAST_EXEC_NS"] = int(
                TimelineSim(_cache[key], no_exec=True).simulate())
        except Exception:
            pass
    nc = _cache[key]
    in_maps = make_in_maps(inputs, prep)
    t0 = _time.time()
    res = run_bass_kernel_spmd(nc, in_maps, core_ids=list(range(NCORES)))
    globals()["LAST_WALL_S"] = _time.time() - t0
    hnT = res.results[0]["hnT_out"].astype(np.float32)
    return head(inputs, hnT)


# revision 5
# speedup vs baseline: 1.3923x; 1.0625x over previous
"""CGCNN forward on 8 Trainium2 NeuronCores — conv layers fully on-device.

Layout: transposed (features on partitions, entities on free), fp16 data.
Edges sorted by dst; core k owns nodes [k*NPAD/8, (k+1)*NPAD/8) and all
edges into them, grouped 128-per-128-node-window (uniform GPW groups per
window for SPMD). h_n replicated; per-layer: AllReduce of BN stats (1KB) +
AllGather of agg slices. Node embedding and graph pooling/head on host.
"""
import sys
sys.path.insert(0, "/opt/trn_rl_repo")
import numpy as np

EPS = 1e-5
NODE_F, EDGE_F, FEAT, NCONV = 92, 41, 64, 3

# problem sizes (overridable for mini tests)
N, E, G = 25000, 400000, 128
NCORES = 8
NPAD = 25600

_cache = {}


def _derived():
    NW = NPAD // 128 // NCORES
    NSLICE = NPAD // NCORES
    ZROW = NPAD
    TROWS = NPAD + 16
    return NW, NSLICE, ZROW, TROWS


# ----------------------------------------------------------------- host prep
def _host_prep(src, dst):
    NW, NSLICE, ZROW, TROWS = _derived()
    order = np.argsort(dst, kind="stable")
    dsts = dst[order]
    srcs = src[order]
    nwin = NPAD // 128
    win = dsts // 128
    wcnt = np.bincount(win, minlength=nwin)
    GPW = int(np.max((wcnt + 127) // 128))
    NGRP = NW * GPW
    EPAD = ((NGRP * 128 + 2047) // 2048) * 2048
    wstart = np.concatenate([[0], np.cumsum(wcnt)])
    src_idx = np.full((NCORES, EPAD), ZROW, np.int16)
    dst_idx = np.full((NCORES, EPAD), ZROW, np.int16)
    dloc = np.full((NCORES, NGRP * 128), -1.0, np.float32)
    eperm = np.full((NCORES, EPAD), -1, np.int64)
    for k in range(NCORES):
        for w in range(NW):
            gw = k * NW + w
            a, b = wstart[gw], wstart[gw + 1]
            ne = b - a
            base = w * GPW * 128
            src_idx[k, base:base + ne] = srcs[a:b].astype(np.int16)
            dst_idx[k, base:base + ne] = dsts[a:b].astype(np.int16)
            dloc[k, base:base + ne] = (dsts[a:b] - gw * 128).astype(np.float32)
            eperm[k, base:base + ne] = order[a:b]

    def wrap16(idx2d):
        out = np.zeros((NCORES, 128, EPAD // 16), np.int16)
        for k in range(NCORES):
            blk = idx2d[k].reshape(EPAD // 16, 16).T
            for c in range(8):
                out[k, c * 16:(c + 1) * 16, :] = blk
        return out

    # dloc per-partition layout: [128 edge-in-group, NGRP]
    dloc_pp = dloc.reshape(NCORES, NGRP, 128).transpose(0, 2, 1).copy()
    return dict(GPW=GPW, EPAD=EPAD, NGRP=NGRP, src_w=wrap16(src_idx),
                dst_w=wrap16(dst_idx), dloc=dloc_pp, eperm=eperm)


# ------------------------------------------------------------ module builder
def _build(EPAD, GPW):
    import concourse.bacc as bacc
    import concourse.mybir as mybir
    import concourse.tile as tile
    from concourse.masks import make_identity

    NW, NSLICE, ZROW, TROWS = _derived()
    f16, f32 = mybir.dt.float16, mybir.dt.float32
    AF = mybir.ActivationFunctionType
    OP = mybir.AluOpType
    X = mybir.AxisListType.X
    NGRP = NW * GPW
    CH = 512
    NCH = EPAD // CH
    GCH = 2048
    NGC = EPAD // GCH
    assert EPAD % GCH == 0 and EPAD % CH == 0 and EPAD >= NGRP * 128
    RG = [[i for i in range(NCORES)]]

    nc = bacc.Bacc("TRN2", target_bir_lowering=False, debug=False,
                   num_devices=NCORES)
    dt_ = nc.dram_tensor
    efT = dt_("efT", [EDGE_F, EPAD], f16, kind="ExternalInput")
    hnT0 = dt_("hnT0", [FEAT, NPAD], f16, kind="ExternalInput")
    srcw = dt_("srcw", [128, EPAD // 16], mybir.dt.int16, kind="ExternalInput")
    dstw = dt_("dstw", [128, EPAD // 16], mybir.dt.int16, kind="ExternalInput")
    dlocd = dt_("dlocd", [128, NGRP], f32, kind="ExternalInput")
    w_ee = dt_("w_ee", [EDGE_F, FEAT], f16, kind="ExternalInput")
    w_emg = dt_("w_emg", [FEAT, NCONV * 128], f16, kind="ExternalInput")
    w_cat = dt_("w_cat", [FEAT, NCONV * 256], f16, kind="ExternalInput")
    gb_e = dt_("gb_e", [FEAT, 2], f32, kind="ExternalInput")
    gb_mg = dt_("gb_mg", [128, NCONV * 2], f32, kind="ExternalInput")
    gb_n = dt_("gb_n", [FEAT, NCONV * 2], f32, kind="ExternalInput")
    npadv = dt_("npadv", [128, 1], f32, kind="ExternalInput")
    hnT_out = dt_("hnT_out", [FEAT, NPAD], f16, kind="ExternalOutput")
    tbl = dt_("tbl", [TROWS, 256], f16)
    zeD = dt_("zeD", [FEAT, EPAD], f16)
    heD = dt_("heD", [FEAT, EPAD], f16)
    cc_in = dt_("cc_in", [128, 2], f32)
    cc_out = dt_("cc_out", [128, 2], f32)
    ag_in = dt_("ag_in", [FEAT * NSLICE], f16)
    ag_out = dt_("ag_out", [NCORES * FEAT * NSLICE], f16)

    def allreduce():
        if NCORES == 1:
            nc.sync.dma_start(cc_out[:], cc_in[:])
        else:
            nc.gpsimd.collective_compute(
                "AllReduce", OP.add, replica_groups=RG,
                ins=[cc_in[:].opt()], outs=[cc_out[:].opt()])

    def allgather():
        if NCORES == 1:
            nc.sync.dma_start(ag_out[:], ag_in[:])
        else:
            nc.gpsimd.collective_compute(
                "AllGather", OP.bypass, replica_groups=RG,
                ins=[ag_in[:].opt()], outs=[ag_out[:].opt()])

    with tile.TileContext(nc) as tc:
        with tc.tile_pool(name="persist", bufs=1) as pp:
            hnT = pp.tile([FEAT, NPAD], f16)
            zmg = pp.tile([128, EPAD], f16)
            srcw_s = pp.tile([128, EPAD // 16], mybir.dt.int16)
            dstw_s = pp.tile([128, EPAD // 16], mybir.dt.int16)
            dloc_s = pp.tile([128, NGRP], f32)
            w_ee_s = pp.tile([EDGE_F, FEAT], f16)
            w_emg_s = pp.tile([FEAT, NCONV * 128], f16)
            w_cat_s = pp.tile([FEAT, NCONV * 256], f16)
            gb_e_s = pp.tile([FEAT, 2], f32)
            gb_mg_s = pp.tile([128, NCONV * 2], f32)
            gb_n_s = pp.tile([FEAT, NCONV * 2], f32)
            npad_s = pp.tile([128, 1], f32)
            ident = pp.tile([128, 128], f16)
            iota_row = pp.tile([128, 128], f16)
            sring = pp.tile([128, 2 * NCH + 2], f32)
            st = pp.tile([128, 8], f32)
            sc_m = pp.tile([128, 1], f32)
            sc_t = pp.tile([128, 1], f32)
            eps_t = pp.tile([128, 1], f32)
            nc.vector.memset(eps_t[:], EPS)
            hpad16 = pp.tile([FEAT, 1], f16)
            cpv = pp.tile([128, 1], f32)
            cp2 = pp.tile([128, 1], f32)

            nc.sync.dma_start(hnT[:], hnT0[:])
            nc.sync.dma_start(srcw_s[:], srcw[:])
            nc.sync.dma_start(dstw_s[:], dstw[:])
            nc.sync.dma_start(dloc_s[:], dlocd[:])
            nc.sync.dma_start(w_ee_s[:], w_ee[:])
            nc.sync.dma_start(w_emg_s[:], w_emg[:])
            nc.sync.dma_start(w_cat_s[:], w_cat[:])
            nc.sync.dma_start(gb_e_s[:], gb_e[:])
            nc.sync.dma_start(gb_mg_s[:], gb_mg[:])
            nc.sync.dma_start(gb_n_s[:], gb_n[:])
            nc.sync.dma_start(npad_s[:], npadv[:])
            make_identity(nc, ident[:])
            ii = pp.tile([128, 128], mybir.dt.int32)
            nc.gpsimd.iota(ii[:], pattern=[[1, 128]], base=0,
                           channel_multiplier=0)
            nc.vector.tensor_copy(iota_row[:], ii[:])

            def bn_affine(p, g_ap, b_ap, inv_n):
                """st[:p,0:2] holds (sum, sumsq); writes sc_m/sc_t[:p]."""
                nc.vector.tensor_scalar(out=st[:p, 2:3], in0=st[:p, 0:1],
                                        scalar1=inv_n, scalar2=None,
                                        op0=OP.mult)
                nc.vector.tensor_scalar(out=st[:p, 3:4], in0=st[:p, 1:2],
                                        scalar1=inv_n, scalar2=None,
                                        op0=OP.mult)
                nc.vector.tensor_tensor(out=st[:p, 4:5], in0=st[:p, 2:3],
                                        in1=st[:p, 2:3], op=OP.mult)
                nc.vector.tensor_tensor(out=st[:p, 3:4], in0=st[:p, 3:4],
                                        in1=st[:p, 4:5], op=OP.subtract)
                nc.scalar.activation(st[:p, 3:4], st[:p, 3:4], AF.Sqrt,
                                     bias=eps_t[:p, :])
                nc.vector.reciprocal(st[:p, 3:4], st[:p, 3:4])
                nc.vector.tensor_tensor(out=sc_m[:p, :], in0=st[:p, 3:4],
                                        in1=g_ap, op=OP.mult)
                nc.vector.tensor_tensor(out=st[:p, 5:6], in0=sc_m[:p, :],
                                        in1=st[:p, 2:3], op=OP.mult)
                nc.vector.tensor_tensor(out=sc_t[:p, :], in0=b_ap,
                                        in1=st[:p, 5:6], op=OP.subtract)

            # ---------------- phase E: z = W_ee.T @ efT, stats, silu
            with tc.tile_pool(name="peb", bufs=2) as sb, \
                 tc.tile_pool(name="pep", bufs=2, space="PSUM") as ps:
                for gc in range(NGC):
                    x = sb.tile([EDGE_F, GCH], f16, tag="x")
                    nc.sync.dma_start(x[:], efT[:, gc * GCH:(gc + 1) * GCH])
                    zs = sb.tile([FEAT, GCH], f16, tag="zs")
                    for s in range(GCH // CH):
                        c = gc * (GCH // CH) + s
                        z = ps.tile([FEAT, CH], f32, tag="z")
                        nc.tensor.matmul(
                            z[:], lhsT=w_ee_s[:],
                            rhs=x[:, s * CH:(s + 1) * CH],
                            start=True, stop=True)
                        nc.scalar.activation(zs[:, s * CH:(s + 1) * CH],
                                             z[:], AF.Identity,
                                             accum_out=sring[:FEAT, c:c + 1])
                        sq = sb.tile([FEAT, CH], f16, tag="sq")
                        nc.scalar.activation(
                            sq[:], zs[:, s * CH:(s + 1) * CH], AF.Square,
                            accum_out=sring[:FEAT, NCH + c:NCH + c + 1])
                    nc.sync.dma_start(zeD[:, gc * GCH:(gc + 1) * GCH], zs[:])
            nc.vector.tensor_reduce(out=st[:FEAT, 0:1],
                                    in_=sring[:FEAT, 0:NCH], op=OP.add,
                                    axis=X)
            nc.vector.tensor_reduce(out=st[:FEAT, 1:2],
                                    in_=sring[:FEAT, NCH:2 * NCH], op=OP.add,
                                    axis=X)
            cci = pp.tile([128, 2], f32)
            nc.vector.memset(cci[:], 0.0)
            nc.vector.tensor_copy(cci[:FEAT, :], st[:FEAT, 0:2])
            nc.sync.dma_start(cc_in[:], cci[:])
            allreduce()
            cco = pp.tile([128, 2], f32)
            nc.sync.dma_start(cco[:], cc_out[:])
            nc.vector.tensor_copy(st[:FEAT, 0:2], cco[:FEAT, :])
            bn_affine(FEAT, gb_e_s[:, 0:1], gb_e_s[:, 1:2], 1.0 / E)
            with tc.tile_pool(name="pe2", bufs=2) as sb:
                for gc in range(NGC):
                    zl = sb.tile([FEAT, GCH], f16, tag="zl")
                    nc.sync.dma_start(zl[:], zeD[:, gc * GCH:(gc + 1) * GCH])
                    nc.vector.tensor_scalar(
                        out=zl[:], in0=zl[:], scalar1=sc_m[:FEAT, :],
                        scalar2=sc_t[:FEAT, :], op0=OP.mult, op1=OP.add)
                    sg = sb.tile([FEAT, GCH], f16, tag="sg")
                    nc.scalar.activation(sg[:], zl[:], AF.Sigmoid)
                    nc.vector.tensor_tensor(out=sg[:], in0=zl[:], in1=sg[:],
                                            op=OP.mult)
                    nc.sync.dma_start(heD[:, gc * GCH:(gc + 1) * GCH], sg[:])
            # hpad = silu(t) (z=0 for pad cols)
            hpadf = pp.tile([FEAT, 1], f32)
            nc.scalar.activation(hpadf[:], sc_t[:FEAT, :], AF.Sigmoid)
            nc.vector.tensor_tensor(out=hpadf[:], in0=hpadf[:],
                                    in1=sc_t[:FEAT, :], op=OP.mult)
            nc.vector.tensor_copy(hpad16[:], hpadf[:])

            # ---------------- conv layers
            for l in range(NCONV):
                lsl = slice(l * 128, (l + 1) * 128)
                # tables
                with tc.tile_pool(name=f"tb{l}", bufs=3) as sb, \
                     tc.tile_pool(name=f"tp{l}", bufs=2, space="PSUM") as ps:
                    for c in range(NPAD // 128):
                        t0 = ps.tile([128, 256], f32, tag="t0")
                        nc.tensor.matmul(
                            t0[:], lhsT=hnT[:, c * 128:(c + 1) * 128],
                            rhs=w_cat_s[:, l * 256:(l + 1) * 256],
                            start=True, stop=True)
                        if c % 8 == 0:
                            stg = sb.tile([128, 8, 256], f16, tag="stg")
                        if c % 2 == 0:
                            nc.vector.tensor_copy(stg[:, c % 8, :], t0[:])
                        else:
                            nc.scalar.activation(stg[:, c % 8, :], t0[:],
                                                 AF.Identity)
                        if c % 8 == 7:
                            nc.sync.dma_start(
                                tbl[(c - 7) * 128:(c + 1) * 128, :].rearrange(
                                    "(b p) c -> p b c", b=8), stg[:])
                    if l == 0:
                        zt = sb.tile([16, 256], f16, tag="zt")
                        nc.vector.memset(zt[:], 0.0)
                        nc.sync.dma_start(tbl[NPAD:TROWS, :], zt[:])
                # pad-edge constant
                with tc.tile_pool(name=f"pc{l}", bufs=1, space="PSUM") as ps:
                    cp = ps.tile([128, 1], f32)
                    nc.tensor.matmul(cp[:], lhsT=w_emg_s[:, lsl],
                                     rhs=hpad16[:], start=True, stop=True)
                    nc.vector.tensor_copy(cpv[:], cp[:])
                    nc.vector.tensor_tensor(out=cp2[:], in0=cpv[:],
                                            in1=cpv[:], op=OP.mult)

                # pass1
                with tc.tile_pool(name=f"p1_{l}", bufs=2) as sb, \
                     tc.tile_pool(name=f"g{l}", bufs=2) as gb, \
                     tc.tile_pool(name=f"q{l}", bufs=2, space="PSUM") as ps:
                    for gc in range(NGC):
                        gs = gb.tile([128, 1, GCH], f16, tag="gs")
                        nc.gpsimd.dma_gather(
                            out_ap=gs[:], in_ap=tbl[:, 0:128],
                            idxs_ap=srcw_s[:, gc * (GCH // 16):
                                           (gc + 1) * (GCH // 16)],
                            num_idxs=GCH, num_idxs_reg=GCH, elem_size=128,
                            elem_step=256, transpose=True,
                            single_packet=False)
                        gd = gb.tile([128, 1, GCH], f16, tag="gd")
                        nc.gpsimd.dma_gather(
                            out_ap=gd[:], in_ap=tbl[:, 128:256],
                            idxs_ap=dstw_s[:, gc * (GCH // 16):
                                           (gc + 1) * (GCH // 16)],
                            num_idxs=GCH, num_idxs_reg=GCH, elem_size=128,
                            elem_step=256, transpose=True,
                            single_packet=False)
                        for s in range(GCH // CH):
                            c = gc * (GCH // CH) + s
                            if s % 2 == 0:
                                he = sb.tile([FEAT, 2 * CH], f16, tag="he")
                                nc.sync.dma_start(
                                    he[:], heD[:, c * CH:(c + 2) * CH])
                            z = ps.tile([128, CH], f32, tag="z")
                            nc.tensor.matmul(
                                z[:], lhsT=w_emg_s[:, lsl],
                                rhs=he[:, (s % 2) * CH:(s % 2 + 1) * CH],
                                start=True, stop=False)
                            nc.tensor.matmul(
                                z[:], lhsT=ident[:],
                                rhs=gs[:, 0, s * CH:(s + 1) * CH],
                                start=False, stop=False)
                            nc.tensor.matmul(
                                z[:], lhsT=ident[:],
                                rhs=gd[:, 0, s * CH:(s + 1) * CH],
                                start=False, stop=True)
                            zd = zmg[:, c * CH:(c + 1) * CH]
                            nc.vector.tensor_scalar(
                                out=zd, in0=z[:], scalar1=1.0, scalar2=0.0,
                                op0=OP.mult, op1=OP.add,
                                accum_out=sring[:, c:c + 1])
                            sq = ps.tile([128, CH], f32, tag="sq")
                            nc.scalar.activation(
                                sq[:], zd, AF.Square,
                                accum_out=sring[:, NCH + c:NCH + c + 1])
                nc.vector.tensor_reduce(out=st[:, 0:1], in_=sring[:, 0:NCH],
                                        op=OP.add, axis=X)
                nc.vector.tensor_reduce(out=st[:, 1:2],
                                        in_=sring[:, NCH:2 * NCH],
                                        op=OP.add, axis=X)
                nc.vector.tensor_tensor(out=st[:, 2:3], in0=cpv[:],
                                        in1=npad_s[:], op=OP.mult)
                nc.vector.tensor_tensor(out=st[:, 0:1], in0=st[:, 0:1],
                                        in1=st[:, 2:3], op=OP.subtract)
                nc.vector.tensor_tensor(out=st[:, 2:3], in0=cp2[:],
                                        in1=npad_s[:], op=OP.mult)
                nc.vector.tensor_tensor(out=st[:, 1:2], in0=st[:, 1:2],
                                        in1=st[:, 2:3], op=OP.subtract)
                cci2 = pp.tile([128, 2], f32, tag="cci2")
                nc.vector.tensor_copy(cci2[:], st[:, 0:2])
                nc.sync.dma_start(cc_in[:], cci2[:])
                allreduce()
                cco2 = pp.tile([128, 2], f32, tag="cco2")
                nc.sync.dma_start(cco2[:], cc_out[:])
                nc.vector.tensor_copy(st[:, 0:2], cco2[:])
                bn_affine(128, gb_mg_s[:, 2 * l:2 * l + 1],
                          gb_mg_s[:, 2 * l + 1:2 * l + 2], 1.0 / E)

                # pass2
                with tc.tile_pool(name=f"p2_{l}", bufs=2) as sb, \
                     tc.tile_pool(name=f"r2{l}", bufs=2, space="PSUM") as ps, \
                     tc.tile_pool(name=f"a2{l}", bufs=2, space="PSUM") as pa:
                    for gc in range(NGC):
                        zc = zmg[:, gc * GCH:(gc + 1) * GCH]
                        nc.scalar.activation(zc, zc, AF.Sigmoid,
                                             bias=sc_t[:], scale=sc_m[:])
                    for gc in range(NGC):
                        zg = zmg[FEAT:128, gc * GCH:(gc + 1) * GCH]
                        nc.scalar.activation(zg, zg, AF.Ln)
                    aggT = pp.tile([FEAT, NSLICE], f16, tag="aggT")
                    QE = 2048
                    ag = None
                    for blk in range(0, NGRP, 4):
                        nb = min(4, NGRP - blk)
                        e0 = blk * 128
                        if e0 % QE == 0:
                            # move Ln(g) half down to partitions 0-63
                            lb = sb.tile([FEAT, QE], f16, tag="lb")
                            nc.sync.dma_start(
                                lb[:], zmg[FEAT:128, e0:e0 + QE])
                        qo = e0 % QE
                        mt = sb.tile([FEAT, CH], f16, tag="mt")
                        nc.vector.tensor_tensor(
                            out=mt[:, :nb * 128],
                            in0=zmg[0:FEAT, e0:e0 + nb * 128],
                            in1=lb[:, qo:qo + nb * 128], op=OP.mult)
                        mn = ps.tile([128, 4 * FEAT], f16, tag="mn")
                        for j in range(nb):
                            nc.tensor.transpose(
                                mn[:, j * FEAT:(j + 1) * FEAT],
                                mt[:, j * 128:(j + 1) * 128],
                                ident[:FEAT, :FEAT])
                        mns = sb.tile([128, 4 * FEAT], f16, tag="mns")
                        nc.vector.tensor_copy(mns[:, :nb * FEAT],
                                              mn[:, :nb * FEAT])
                        for j in range(nb):
                            grp = blk + j
                            w, g = grp // GPW, grp % GPW
                            if g == 0:
                                ag = pa.tile([FEAT, 128], f32, tag="ag")
                            oh = sb.tile([128, 128], f16, tag="oh")
                            nc.vector.tensor_scalar(
                                out=oh[:], in0=iota_row[:],
                                scalar1=dloc_s[:, grp:grp + 1],
                                scalar2=-1.0, op0=OP.is_equal, op1=OP.mult)
                            nc.tensor.matmul(
                                ag[:], lhsT=mns[:, j * FEAT:(j + 1) * FEAT],
                                rhs=oh[:], start=(g == 0),
                                stop=(g == GPW - 1))
                            if g == GPW - 1:
                                nc.vector.tensor_copy(
                                    aggT[:, w * 128:(w + 1) * 128], ag[:])
                    nc.sync.dma_start(
                        ag_in[:].rearrange("(a b) -> a b", a=FEAT), aggT[:])
                allgather()
                # agg stats + h_n update
                HS = NSLICE // 2
                NPC = 2 * NCORES  # pieces of [FEAT, HS]
                agr = ag_out[:].rearrange("(c f hh h) -> c f hh h",
                                          c=NCORES, f=FEAT, hh=2)
                with tc.tile_pool(name=f"u{l}", bufs=2) as sb:
                    for k in range(NPC):
                        t = sb.tile([FEAT, HS], f16, tag="agld")
                        nc.sync.dma_start(t[:], agr[k // 2][:, k % 2, :])
                        d1 = sb.tile([FEAT, HS], f16, tag="dsink")
                        nc.vector.tensor_scalar(
                            out=d1[:], in0=t[:], scalar1=1.0, scalar2=0.0,
                            op0=OP.mult, op1=OP.add,
                            accum_out=sring[:FEAT, k:k + 1])
                        d2 = sb.tile([FEAT, HS], f16, tag="dsink")
                        nc.scalar.activation(
                            d2[:], t[:], AF.Square,
                            accum_out=sring[:FEAT, NPC + k:NPC + k + 1])
                    nc.vector.tensor_reduce(
                        out=st[:FEAT, 0:1], in_=sring[:FEAT, 0:NPC],
                        op=OP.add, axis=X)
                    nc.vector.tensor_reduce(
                        out=st[:FEAT, 1:2], in_=sring[:FEAT, NPC:2 * NPC],
                        op=OP.add, axis=X)
                    bn_affine(FEAT, gb_n_s[:, 2 * l:2 * l + 1],
                              gb_n_s[:, 2 * l + 1:2 * l + 2], 1.0 / N)
                    for k in range(NPC):
                        t = sb.tile([FEAT, HS], f16, tag="agld")
                        nc.sync.dma_start(t[:], agr[k // 2][:, k % 2, :])
                        tmp = sb.tile([FEAT, HS], f16, tag="tmp")
                        nc.vector.tensor_scalar(
                            out=tmp[:], in0=t[:], scalar1=sc_m[:FEAT, :],
                            scalar2=sc_t[:FEAT, :], op0=OP.mult, op1=OP.add)
                        hsl = hnT[:, k * HS:(k + 1) * HS]
                        nc.vector.tensor_tensor(out=tmp[:], in0=tmp[:],
                                                in1=hsl, op=OP.add)
                        nc.scalar.activation(hsl, tmp[:], AF.Sigmoid)
            nc.sync.dma_start(hnT_out[:], hnT[:])
    nc.compile()
    return nc


# ------------------------------------------------------------------- kernel
def _silu(x):
    return x / (1.0 + np.exp(-x))


def _bn(x, g, b):
    return g * (x - x.mean(0)) / np.sqrt(x.var(0) + EPS) + b


def make_in_maps(inputs, prep):
    """Host-side marshaling: returns (in_maps, host_ctx)."""
    f32 = lambda k: np.asarray(inputs[k], np.float32)
    node_feats = f32("node_feats")
    edge_feats = f32("edge_feats")
    EPAD = prep["EPAD"]

    h_n0 = _silu(_bn(node_feats @ f32("W_ne"), f32("g_ne"), f32("be_ne")))
    hnT0 = np.zeros((FEAT, NPAD), np.float16)
    hnT0[:, :N] = h_n0.T.astype(np.float16)

    Wm, Wg = f32("Wm"), f32("Wg")
    w_ee = f32("W_ee").astype(np.float16)
    w_emg = np.concatenate(
        [np.concatenate([Wm[l][2 * FEAT:], Wg[l][2 * FEAT:]], 1)
         for l in range(NCONV)], 1).astype(np.float16)
    w_cat = np.concatenate(
        [np.concatenate([Wm[l][:FEAT], Wg[l][:FEAT],
                         Wm[l][FEAT:2 * FEAT], Wg[l][FEAT:2 * FEAT]], 1)
         for l in range(NCONV)], 1).astype(np.float16)
    gb_e = np.ascontiguousarray(
        np.stack([f32("g_ee"), f32("be_ee")], 1).astype(np.float32))
    gb_mg = np.zeros((128, NCONV * 2), np.float32)
    gb_n = np.zeros((FEAT, NCONV * 2), np.float32)
    for l in range(NCONV):
        gb_mg[:FEAT, 2 * l] = f32("gm")[l]
        gb_mg[FEAT:, 2 * l] = -f32("gg")[l]
        gb_mg[:FEAT, 2 * l + 1] = f32("bem")[l]
        gb_mg[FEAT:, 2 * l + 1] = -f32("beg")[l]
        gb_n[:, 2 * l] = f32("gn")[l]
        gb_n[:, 2 * l + 1] = f32("ben")[l]

    in_maps = []
    for k in range(NCORES):
        efT = np.zeros((EDGE_F, EPAD), np.float16)
        valid = prep["eperm"][k] >= 0
        efT[:, valid] = edge_feats[prep["eperm"][k][valid]].T.astype(
            np.float16)
        npadv = np.full((128, 1), float(EPAD - valid.sum()), np.float32)
        in_maps.append(dict(
            efT=efT, hnT0=hnT0, srcw=prep["src_w"][k], dstw=prep["dst_w"][k],
            dlocd=np.ascontiguousarray(prep["dloc"][k]), w_ee=w_ee,
            w_emg=w_emg, w_cat=w_cat, gb_e=gb_e, gb_mg=gb_mg, gb_n=gb_n,
            npadv=npadv))
    return in_maps


def head(inputs, hnT):
    f32 = lambda k: np.asarray(inputs[k], np.float32)
    n2g = np.asarray(inputs["node2graph"], np.int64)
    h_n = hnT[:, :N].T.astype(np.float32)
    sums = np.zeros((G, FEAT), np.float32)
    np.add.at(sums, n2g, h_n)
    cnt = np.bincount(n2g, minlength=G).astype(np.float32)[:, None]
    pooled = sums / np.maximum(cnt, 1.0)
    h = _silu(_bn(pooled @ f32("W_fc") + f32("b_fc"), f32("g_fc"),
                  f32("be_fc")))
    return (h @ f32("W_out") + f32("b_out")).astype(np.float32)


def kernel(**inputs):
    import time as _time
    from concourse.bass_utils import run_bass_kernel_spmd

    src = np.asarray(inputs["src"], np.int64)
    dst = np.asarray(inputs["dst"], np.int64)
    prep = _host_prep(src, dst)
    key = ("nc", prep["EPAD"], prep["GPW"])
    if key not in _cache:
        _cache[key] = _build(prep["EPAD"], prep["GPW"])
        try:
            from concourse.timeline_sim import TimelineSim
            globals()["LAST_EXEC_NS"] = int(
                TimelineSim(_cache[key], no_exec=True).simulate())
        except Exception:
            pass
    nc = _cache[key]
    in_maps = make_in_maps(inputs, prep)
    t0 = _time.time()
    res = run_bass_kernel_spmd(nc, in_maps, core_ids=list(range(NCORES)))
    globals()["LAST_WALL_S"] = _time.time() - t0
    hnT = res.results[0]["hnT_out"].astype(np.float32)
    return head(inputs, hnT)
